# revision 25
# baseline (speedup 1.0000x reference)
"""Trainium2 Bass kernel for the GIN ActorCritic forward pass.

Shards batch-parallel over 8 NeuronCores (4 graphs each). Host-side
preprocessing: transpose+bf16-cast adjacency, build one-hot candidate
gather matrix (torch.unique semantics) with graph_pool packed as an
extra column, fold actor bias b3 + mask into an additive score mask.
"""
import sys
import types

sys.path.insert(0, "/opt/trn_rl_repo")

import numpy as np
import ml_dtypes

import concourse.bass as bass
import concourse.mybir as mybir
import concourse.tile as tile
from concourse.vector_clock import ScopedClock
from concourse.masks import make_identity
from concourse.bass_utils import run_bass_kernel_spmd

BF16 = mybir.dt.bfloat16
F32 = mybir.dt.float32
F32R = mybir.dt.float32r
AF = mybir.ActivationFunctionType
ALU = mybir.AluOpType

B, N, D, H, HA = 32, 1000, 8, 256, 64
NJ, NM = 100, 10
N_CORES = 8
BPC = B // N_CORES  # 4 graphs per core
SC = 104  # ST columns: 100 one-hot cand cols + col 100 = graph_pool + pad
NT = 8  # node tiles of 128 (last is 104)
TS = [128] * 7 + [104]
CH = [(0, 512), (512, 488)]  # free-dim chunks for GIN stages
CHA = [(0, 500), (500, 500)]  # actor chunks (aligned to cand groups of 10)
NEG = -1.0e30

_nbf16 = ml_dtypes.bfloat16


# ---------------------------------------------------------------------------
# Tile drain patch: walrus in this image rejects >2 sync waits on a CTRL
# drain; split the final global-clock drain into one-wait-per-drain chain.
def _patched_drain_and_barrier(self, tick_clock, wait_clock):
    nc = self.nc
    drain_inst = nc.sync.drain()
    wait_clock.add_sem_waits(
        drain_inst.ins, ScopedClock({None: tick_clock.global_clock})
    )
    waits = list(drain_inst.ins.sync_info.on_wait or [])
    if len(waits) > 1:
        drain_inst.ins.sync_info.on_wait = waits[:1]
        for w in waits[1:]:
            d = nc.sync.drain()
            d.ins.sync_info = mybir.SyncInfo(on_wait=[w], on_update=[])
    nc.all_engine_barrier()
    popped = nc._tile_sem_poison_stack.pop()
    assert popped is self._sem_poison
    nc.clear_and_free_semaphores(list(self.sems.allocated().values()))
    nc.all_engine_barrier()


tile.TileContext._drain_and_barrier = _patched_drain_and_barrier

MAX_WAITS = 1


def _split_sync_waits(nc, max_waits=MAX_WAITS):
    """walrus in this image encodes at most `max_waits` sem-waits per
    instruction; hoist the excess into same-engine NoOps placed just
    before the instruction."""
    n_split = 0
    for f in nc.m.functions:
        for bb in f.blocks:
            insts = list(bb.instructions)
            out = []
            changed = False
            for inst in insts:
                si = inst.sync_info
                waits = list(si.on_wait) if (si is not None and si.on_wait) else []
                if len(waits) > max_waits:
                    changed = True
                    extra = waits[: len(waits) - max_waits]
                    for i in range(0, len(extra), max_waits):
                        chunk = extra[i : i + max_waits]
                        nop = mybir.InstNoOp(
                            name=f"I-wsplit-{n_split}",
                            engine=inst.engine,
                            ins=[],
                            outs=[],
                            sync_info=mybir.SyncInfo(on_wait=chunk, on_update=[]),
                        )
                        n_split += 1
                        out.append(nop)
                    si.on_wait = waits[len(waits) - max_waits :]
                out.append(inst)
            if changed:
                bb.instructions = out
    return n_split


def _install_ntff_shim():
    """Provide the missing antenv.axon_hooks so trace=True works (test.py)."""
    if "antenv.axon_hooks" in sys.modules:
        return
    mod = types.ModuleType("antenv.axon_hooks")
    mod._hook = None
    mod.set_axon_ntff_profile_hook = lambda h: setattr(mod, "_hook", h)
    mod.get_axon_ntff_profile_hook = lambda: mod._hook
    sys.modules["antenv.axon_hooks"] = mod
    import antenv

    antenv.axon_hooks = mod
    try:
        from trn_agent_boot.trn_boot import _ntff_profile_via_ctypes

        mod.set_axon_ntff_profile_hook(
            _ntff_profile_via_ctypes("/opt/axon/libaxon_pjrt.so")
        )
    except Exception:
        pass


def _bcast_ap(ap, count=128):
    """Partition-broadcast a [1,1]-style dram element to `count` partitions."""
    return bass.AP(tensor=ap.tensor, offset=ap.offset, ap=[[0, count]] + list(ap.ap))


def _rep10_ap(ap):
    """Append an inner stride-0 dim of 10 (repeat_interleave along free)."""
    return bass.AP(
        tensor=ap.tensor, offset=ap.offset, ap=list(ap.ap) + [[0, NM]]
    )


def _build_nc():
    nc = bass.Bass()

    # --- per-core sharded inputs -----------------------------------------
    adjT_e = nc.declare_dram_parameter("adjT", [BPC, N, N], BF16, isOutput=False)
    x_e = nc.declare_dram_parameter("x", [BPC, N, D], BF16, isOutput=False)
    xT_e = nc.declare_dram_parameter("xT", [BPC, D, N], BF16, isOutput=False)
    ST_e = nc.declare_dram_parameter("ST", [BPC, N, SC], BF16, isOutput=False)
    machT_e = nc.declare_dram_parameter("machT", [BPC, 4, N], BF16, isOutput=False)
    mneg_e = nc.declare_dram_parameter("maskneg", [BPC, 1, N], F32, isOutput=False)
    # --- replicated weights ----------------------------------------------
    eps_e = nc.declare_dram_parameter("eps", [2, 1], F32, isOutput=False)
    w1a_e = nc.declare_dram_parameter("w1a", [D, H], F32R, isOutput=False)
    w1b_e = nc.declare_dram_parameter("w1b", [2, 128, H], F32R, isOutput=False)
    w2a_e = nc.declare_dram_parameter("w2a", [2, 128, H], F32R, isOutput=False)
    w2b_e = nc.declare_dram_parameter("w2b", [2, 128, H], F32R, isOutput=False)
    b1a_e = nc.declare_dram_parameter("b1a", [128, 2], F32, isOutput=False)
    b1b_e = nc.declare_dram_parameter("b1b", [128, 2], F32, isOutput=False)
    b2a_e = nc.declare_dram_parameter("b2a", [128, 2], F32, isOutput=False)
    b2b_e = nc.declare_dram_parameter("b2b", [128, 2], F32, isOutput=False)
    wa1c_e = nc.declare_dram_parameter("wa1c", [2, 128, HA], BF16, isOutput=False)
    wa1p_e = nc.declare_dram_parameter("wa1p", [2, 128, HA], F32, isOutput=False)
    wa1m_e = nc.declare_dram_parameter("wa1m", [4, HA], BF16, isOutput=False)
    ba1_e = nc.declare_dram_parameter("ba1", [HA, 1], F32, isOutput=False)
    wa2_e = nc.declare_dram_parameter("wa2", [HA, HA], BF16, isOutput=False)
    ba2_e = nc.declare_dram_parameter("ba2", [HA, 1], F32, isOutput=False)
    wa3_e = nc.declare_dram_parameter("wa3", [HA, 1], BF16, isOutput=False)
    wc1_e = nc.declare_dram_parameter("wc1", [2, 128, HA], F32, isOutput=False)
    bc1_e = nc.declare_dram_parameter("bc1", [HA, 1], F32, isOutput=False)
    wc2_e = nc.declare_dram_parameter("wc2", [HA, HA], F32, isOutput=False)
    bc2_e = nc.declare_dram_parameter("bc2", [HA, 1], F32, isOutput=False)
    wc3_e = nc.declare_dram_parameter("wc3", [HA, 1], F32, isOutput=False)
    bc3_e = nc.declare_dram_parameter("bc3", [1, 1], F32, isOutput=False)
    out_e = nc.declare_dram_parameter("out", [BPC, 1001], F32, isOutput=True)

    from contextlib import ExitStack

    with tile.TileContext(nc) as tc, ExitStack() as ctx:
        wp = ctx.enter_context(tc.tile_pool(name="wp", bufs=1))
        ap_ = ctx.enter_context(tc.tile_pool(name="adj", bufs=3))
        sp = ctx.enter_context(tc.tile_pool(name="small", bufs=3))
        hp = ctx.enter_context(tc.tile_pool(name="acts", bufs=2))
        pmm = ctx.enter_context(tc.tile_pool(name="pmm", bufs=5, space="PSUM"))
        ptp = ctx.enter_context(tc.tile_pool(name="ptp", bufs=2, space="PSUM"))
        psm = ctx.enter_context(tc.tile_pool(name="psm", bufs=1, space="PSUM"))

        # ---- constants & weights (loaded once) --------------------------
        ident = wp.tile([128, 128], BF16)
        make_identity(nc, ident[:, :])

        eps0 = wp.tile([128, 1], F32, tag="eps0")
        eps1 = wp.tile([128, 1], F32, tag="eps1")
        e_ap = eps_e[:, :]
        nc.sync.dma_start(
            out=eps0[:, :],
            in_=bass.AP(tensor=e_ap.tensor, offset=e_ap.offset, ap=[[0, 128], [1, 1]]),
        )
        nc.sync.dma_start(
            out=eps1[:, :],
            in_=bass.AP(
                tensor=e_ap.tensor, offset=e_ap.offset + 1, ap=[[0, 128], [1, 1]]
            ),
        )
        # 1 + eps
        nc.scalar.add(out=eps0[:, :], in_=eps0[:, :], add=1.0)
        nc.scalar.add(out=eps1[:, :], in_=eps1[:, :], add=1.0)

        w1a = wp.tile([D, H], F32R, tag="w1a")
        nc.sync.dma_start(out=w1a[:, :], in_=w1a_e[:, :])
        gin_w = {}
        for nm, ext in (("w1b", w1b_e), ("w2a", w2a_e), ("w2b", w2b_e)):
            t = wp.tile([128, 2, H], F32R, tag=nm)
            for k in range(2):
                nc.sync.dma_start(out=t[:, k, :], in_=ext[k])
            gin_w[nm] = t
        gin_b = {}
        for nm, ext in (
            ("b1a", b1a_e),
            ("b1b", b1b_e),
            ("b2a", b2a_e),
            ("b2b", b2b_e),
        ):
            t = wp.tile([128, 2], F32, tag=nm)
            nc.sync.dma_start(out=t[:, :], in_=ext[:, :])
            gin_b[nm] = t
        wa1c = wp.tile([128, 2, HA], BF16, tag="wa1c")
        wa1p = wp.tile([128, 2, HA], F32, tag="wa1p")
        wc1 = wp.tile([128, 2, HA], F32, tag="wc1")
        for t, ext in ((wa1c, wa1c_e), (wa1p, wa1p_e), (wc1, wc1_e)):
            for k in range(2):
                nc.sync.dma_start(out=t[:, k, :], in_=ext[k])
        wa1m = wp.tile([4, HA], BF16, tag="wa1m")
        nc.sync.dma_start(out=wa1m[:, :], in_=wa1m_e[:, :])
        wa2 = wp.tile([HA, HA], BF16, tag="wa2")
        nc.sync.dma_start(out=wa2[:, :], in_=wa2_e[:, :])
        wa3 = wp.tile([HA, 1], BF16, tag="wa3")
        nc.sync.dma_start(out=wa3[:, :], in_=wa3_e[:, :])
        wc2 = wp.tile([HA, HA], F32, tag="wc2")
        nc.sync.dma_start(out=wc2[:, :], in_=wc2_e[:, :])
        wc3 = wp.tile([HA, 1], F32, tag="wc3")
        nc.sync.dma_start(out=wc3[:, :], in_=wc3_e[:, :])
        ba1 = wp.tile([HA, 1], F32, tag="ba1")
        nc.sync.dma_start(out=ba1[:, :], in_=ba1_e[:, :])
        ba2 = wp.tile([HA, 1], F32, tag="ba2")
        nc.sync.dma_start(out=ba2[:, :], in_=ba2_e[:, :])
        bc1 = wp.tile([HA, 1], F32, tag="bc1")
        nc.sync.dma_start(out=bc1[:, :], in_=bc1_e[:, :])
        bc2 = wp.tile([HA, 1], F32, tag="bc2")
        nc.sync.dma_start(out=bc2[:, :], in_=bc2_e[:, :])
        bc3 = wp.tile([1, 1], F32, tag="bc3")
        nc.sync.dma_start(out=bc3[:, :], in_=bc3_e[:, :])

        for b in range(BPC):
            # ---- load per-batch inputs ----------------------------------
            adjT = ap_.tile([128, 7, N], BF16, tag="adjT")
            adjTt = ap_.tile([128, N], BF16, tag="adjTt")
            nc.sync.dma_start(
                out=adjT[:, :, :],
                in_=adjT_e[b, 0:896, :].rearrange("(j p) i -> p j i", p=128),
            )
            nc.sync.dma_start(out=adjTt[0:104, :], in_=adjT_e[b, 896:1000, :])

            x_sb = sp.tile([128, 7, D], BF16, tag="x")
            x_tl = sp.tile([128, D], BF16, tag="xt")
            nc.sync.dma_start(
                out=x_sb[:, :, :],
                in_=x_e[b, 0:896, :].rearrange("(j p) d -> p j d", p=128),
            )
            nc.sync.dma_start(out=x_tl[0:104, :], in_=x_e[b, 896:1000, :])

            ST_sb = sp.tile([128, 7, SC], BF16, tag="ST")
            ST_tl = sp.tile([128, SC], BF16, tag="STt")
            nc.sync.dma_start(
                out=ST_sb[:, :, :],
                in_=ST_e[b, 0:896, :].rearrange("(j p) c -> p j c", p=128),
            )
            nc.sync.dma_start(out=ST_tl[0:104, :], in_=ST_e[b, 896:1000, :])

            xT_sb = sp.tile([D, N], BF16, tag="xT")
            nc.sync.dma_start(out=xT_sb[:, :], in_=xT_e[b])
            machT = sp.tile([4, N], BF16, tag="machT")
            nc.sync.dma_start(out=machT[:, :], in_=machT_e[b])
            mneg = sp.tile([1, N], F32, tag="mneg")
            nc.sync.dma_start(out=mneg[:, :], in_=mneg_e[b])

            def adjT_blk(j, c0, cn):
                if j < 7:
                    return adjT[:, j, c0 : c0 + cn]
                return adjTt[0:104, c0 : c0 + cn]

            def x_blk(j):
                if j < 7:
                    return x_sb[:, j, :]
                return x_tl[0:104, :]

            def ST_blk(j):
                if j < 7:
                    return ST_sb[:, j, 0:SC]
                return ST_tl[0:104, 0:SC]

            # ---- GIN layer 1 --------------------------------------------
            # pooled1T[d, i] = sum_j x[j, d] * adjT[j, i]  (+ (1+eps0)*xT)
            p1T = hp.tile([D, N], F32R, tag="p1T")
            for c0, cn in CH:
                q = pmm.tile([D, 512], F32, tag="mm")
                for j in range(NT):
                    nc.tensor.matmul(
                        q[0:D, 0:cn],
                        lhsT=x_blk(j),
                        rhs=adjT_blk(j, c0, cn),
                        start=(j == 0),
                        stop=(j == NT - 1),
                    )
                nc.vector.scalar_tensor_tensor(
                    out=p1T[:, c0 : c0 + cn],
                    in0=xT_sb[:, c0 : c0 + cn],
                    scalar=eps0[0:D, :],
                    in1=q[0:D, 0:cn],
                    op0=ALU.mult,
                    op1=ALU.add,
                )

            # relu1T = relu(w1a^T @ p1T + b1a)
            r1T = hp.tile([128, 2, N], F32R, tag="r1T")
            for m in range(2):
                for c0, cn in CH:
                    q = pmm.tile([128, 512], F32, tag="mm")
                    nc.tensor.matmul(
                        q[:, 0:cn],
                        lhsT=w1a[:, m * 128 : (m + 1) * 128],
                        rhs=p1T[:, c0 : c0 + cn],
                        start=True,
                        stop=True,
                    )
                    nc.scalar.activation(
                        out=r1T[:, m, c0 : c0 + cn],
                        in_=q[:, 0:cn],
                        func=AF.Relu,
                        bias=gin_b["b1a"][:, m : m + 1],
                    )

            # h1T = relu(w1b^T @ r1T + b1b); h1 node-major via PE transpose
            h1T = hp.tile([128, 2, N], BF16, tag="h1T")
            for m in range(2):
                for c0, cn in CH:
                    q = pmm.tile([128, 512], F32, tag="mm")
                    for k in range(2):
                        nc.tensor.matmul(
                            q[:, 0:cn],
                            lhsT=gin_w["w1b"][:, k, m * 128 : (m + 1) * 128],
                            rhs=r1T[:, k, c0 : c0 + cn],
                            start=(k == 0),
                            stop=(k == 1),
                        )
                    nc.scalar.activation(
                        out=h1T[:, m, c0 : c0 + cn],
                        in_=q[:, 0:cn],
                        func=AF.Relu,
                        bias=gin_b["b1b"][:, m : m + 1],
                    )
            h1nm = hp.tile([128, NT, H], BF16, tag="h1nm")
            for m in range(2):
                for j in range(NT):
                    tsz = TS[j]
                    tq = ptp.tile([128, 128], BF16, tag="tp")
                    nc.tensor.transpose(
                        tq[0:tsz, 0:128],
                        in_=h1T[:, m, j * 128 : j * 128 + tsz],
                        identity=ident[:, :],
                    )
                    nc.vector.tensor_copy(
                        out=h1nm[0:tsz, j, m * 128 : (m + 1) * 128],
                        in_=tq[0:tsz, 0:128],
                    )

            # ---- GIN layer 2 --------------------------------------------
            p2T = hp.tile([128, 2, N], F32R, tag="p2T")
            for m in range(2):
                for c0, cn in CH:
                    q = pmm.tile([128, 512], F32, tag="mm")
                    for j in range(NT):
                        nc.tensor.matmul(
                            q[:, 0:cn],
                            lhsT=h1nm[0 : TS[j], j, m * 128 : (m + 1) * 128],
                            rhs=adjT_blk(j, c0, cn),
                            start=(j == 0),
                            stop=(j == NT - 1),
                        )
                    nc.vector.scalar_tensor_tensor(
                        out=p2T[:, m, c0 : c0 + cn],
                        in0=h1T[:, m, c0 : c0 + cn],
                        scalar=eps1[:, :],
                        in1=q[:, 0:cn],
                        op0=ALU.mult,
                        op1=ALU.add,
                    )

            r2T = hp.tile([128, 2, N], F32R, tag="r2T")
            for m in range(2):
                for c0, cn in CH:
                    q = pmm.tile([128, 512], F32, tag="mm")
                    for k in range(2):
                        nc.tensor.matmul(
                            q[:, 0:cn],
                            lhsT=gin_w["w2a"][:, k, m * 128 : (m + 1) * 128],
                            rhs=p2T[:, k, c0 : c0 + cn],
                            start=(k == 0),
                            stop=(k == 1),
                        )
                    nc.scalar.activation(
                        out=r2T[:, m, c0 : c0 + cn],
                        in_=q[:, 0:cn],
                        func=AF.Relu,
                        bias=gin_b["b2a"][:, m : m + 1],
                    )

            h2T = hp.tile([128, 2, N], BF16, tag="h2T")
            for m in range(2):
                for c0, cn in CH:
                    q = pmm.tile([128, 512], F32, tag="mm")
                    for k in range(2):
                        nc.tensor.matmul(
                            q[:, 0:cn],
                            lhsT=gin_w["w2b"][:, k, m * 128 : (m + 1) * 128],
                            rhs=r2T[:, k, c0 : c0 + cn],
                            start=(k == 0),
                            stop=(k == 1),
                        )
                    nc.scalar.activation(
                        out=h2T[:, m, c0 : c0 + cn],
                        in_=q[:, 0:cn],
                        func=AF.Relu,
                        bias=gin_b["b2b"][:, m : m + 1],
                    )
            h2nm = hp.tile([128, NT, H], BF16, tag="h2nm")
            for m in range(2):
                for j in range(NT):
                    tsz = TS[j]
                    tq = ptp.tile([128, 128], BF16, tag="tp")
                    nc.tensor.transpose(
                        tq[0:tsz, 0:128],
                        in_=h2T[:, m, j * 128 : j * 128 + tsz],
                        identity=ident[:, :],
                    )
                    nc.vector.tensor_copy(
                        out=h2nm[0:tsz, j, m * 128 : (m + 1) * 128],
                        in_=tq[0:tsz, 0:128],
                    )

            # ---- candidate gather + graph pool (one matmul) --------------
            # cfT[d, c] = sum_n h2[n, d] * ST[n, c]; col 100 = h_pooled
            cfT = hp.tile([128, 2, SC], BF16, tag="cfT")
            hp32 = hp.tile([128, 2], F32, tag="hp32")
            for m in range(2):
                q = ptp.tile([128, SC], F32, tag="tp")
                for j in range(NT):
                    nc.tensor.matmul(
                        q[:, 0:SC],
                        lhsT=h2nm[0 : TS[j], j, m * 128 : (m + 1) * 128],
                        rhs=ST_blk(j),
                        start=(j == 0),
                        stop=(j == NT - 1),
                    )
                nc.scalar.copy(out=cfT[:, m, :], in_=q[:, 0:SC])
                nc.scalar.copy(out=hp32[:, m : m + 1], in_=q[:, 100:101])

            # ---- actor bias u = wa1p^T @ h_pooled + ba1 ------------------
            qu = psm.tile([HA, 1], F32, tag="qu")
            for k in range(2):
                nc.tensor.matmul(
                    qu[:, :],
                    lhsT=wa1p[:, k, :],
                    rhs=hp32[:, k : k + 1],
                    start=(k == 0),
                    stop=(k == 1),
                )
            ua = hp.tile([HA, 1], F32, tag="ua")
            nc.vector.tensor_add(out=ua[:, :], in0=qu[:, :], in1=ba1[:, :])

            # ---- actor layer 1: cand(rep10) + mach + bias ----------------
            a1T = hp.tile([HA, N], BF16, tag="a1T")
            for c0, cn in CHA:
                q = pmm.tile([HA, 512], F32, tag="mm")
                for k in range(2):
                    src = cfT[:, k, c0 // NM : (c0 + cn) // NM]
                    nc.tensor.matmul(
                        q[:, 0:cn],
                        lhsT=wa1c[:, k, :],
                        rhs=_rep10_ap(src),
                        start=(k == 0),
                        stop=False,
                    )
                nc.tensor.matmul(
                    q[:, 0:cn],
                    lhsT=wa1m[:, :],
                    rhs=machT[:, c0 : c0 + cn],
                    start=False,
                    stop=True,
                )
                nc.scalar.activation(
                    out=a1T[:, c0 : c0 + cn],
                    in_=q[:, 0:cn],
                    func=AF.Tanh,
                    bias=ua[:, :],
                )

            # ---- actor layer 2 ------------------------------------------
            a2T = hp.tile([HA, N], BF16, tag="a2T")
            for c0, cn in CHA:
                q = pmm.tile([HA, 512], F32, tag="mm")
                nc.tensor.matmul(
                    q[:, 0:cn],
                    lhsT=wa2[:, :],
                    rhs=a1T[:, c0 : c0 + cn],
                    start=True,
                    stop=True,
                )
                nc.scalar.activation(
                    out=a2T[:, c0 : c0 + cn],
                    in_=q[:, 0:cn],
                    func=AF.Tanh,
                    bias=ba2[:, :],
                )

            # ---- scores + mask (+ba3 folded into maskneg) ----------------
            sT = hp.tile([1, N], F32, tag="sT")
            for c0, cn in CHA:
                q = pmm.tile([1, 512], F32, tag="mm")
                nc.tensor.matmul(
                    q[0:1, 0:cn],
                    lhsT=wa3[:, :],
                    rhs=a2T[:, c0 : c0 + cn],
                    start=True,
                    stop=True,
                )
                nc.vector.tensor_add(
                    out=sT[:, c0 : c0 + cn],
                    in0=q[0:1, 0:cn],
                    in1=mneg[:, c0 : c0 + cn],
                )

            # ---- masked softmax over the 1000 candidates -----------------
            nmx = hp.tile([1, 1], F32, tag="nmx")
            nc.vector.reduce_max(out=nmx[:, :], in_=sT[:, :], axis=mybir.AxisListType.X, negate=True)
            esb = hp.tile([1, N], F32, tag="esb")
            ssum = hp.tile([1, 1], F32, tag="ssum")
            nc.scalar.activation(
                out=esb[:, :],
                in_=sT[:, :],
                func=AF.Exp,
                bias=nmx[:, :],
                accum_out=ssum[:, :],
            )
            rsum = hp.tile([1, 1], F32, tag="rsum")
            nc.vector.reciprocal(out=rsum[:, :], in_=ssum[:, :])
            pi = hp.tile([1, N], F32, tag="pi")
            nc.vector.tensor_scalar_mul(pi[:, :], in0=esb[:, :], scalar1=rsum[:, :])
            nc.sync.dma_start(out=out_e[b : b + 1, 0:1000], in_=pi[:, :])

            # ---- critic head --------------------------------------------
            qc1 = psm.tile([HA, 1], F32, tag="qu")
            for k in range(2):
                nc.tensor.matmul(
                    qc1[:, :],
                    lhsT=wc1[:, k, :],
                    rhs=hp32[:, k : k + 1],
                    start=(k == 0),
                    stop=(k == 1),
                )
            c1 = hp.tile([HA, 1], F32, tag="c1")
            nc.scalar.activation(out=c1[:, :], in_=qc1[:, :], func=AF.Tanh, bias=bc1[:, :])
            qc2 = psm.tile([HA, 1], F32, tag="qu")
            nc.tensor.matmul(qc2[:, :], lhsT=wc2[:, :], rhs=c1[:, :], start=True, stop=True)
            c2 = hp.tile([HA, 1], F32, tag="c2")
            nc.scalar.activation(out=c2[:, :], in_=qc2[:, :], func=AF.Tanh, bias=bc2[:, :])
            qv = psm.tile([1, 1], F32, tag="qu")
            nc.tensor.matmul(qv[:, :], lhsT=wc3[:, :], rhs=c2[:, :], start=True, stop=True)
            v = hp.tile([1, 1], F32, tag="v")
            nc.scalar.activation(out=v[:, :], in_=qv[:, :], func=AF.Identity, bias=bc3[:, :])
            nc.sync.dma_start(out=out_e[b : b + 1, 1000:1001], in_=v[:, :])

    _split_sync_waits(nc)
    return nc


_NC_CACHE = {}


def _get_nc():
    if "nc" not in _NC_CACHE:
        _NC_CACHE["nc"] = _build_nc()
    return _NC_CACHE["nc"]


def _leaf(a):
    return np.asarray(a)


def _prep_inputs(inputs):
    x = _leaf(inputs["x"]).astype(np.float32)
    adj = _leaf(inputs["adj_matrix"]).astype(np.float32)
    gpool = _leaf(inputs["graph_pool"]).astype(np.float32)
    cand = _leaf(inputs["candidate"])
    mask = _leaf(inputs["mask"])
    mach = _leaf(inputs["machine_feat"]).astype(np.float32)
    gin_params = [[(_leaf(w), _leaf(bb)) for (w, bb) in layer] for layer in inputs["gin_params"]]
    eps = _leaf(inputs["eps"]).astype(np.float32)
    actor = [(_leaf(w), _leaf(bb)) for (w, bb) in inputs["actor_params"]]
    critic = [(_leaf(w), _leaf(bb)) for (w, bb) in inputs["critic_params"]]

    # torch.unique semantics (jnp.unique size=NJ fill=0): sorted unique,
    # truncated/padded to NJ
    cand0 = cand[:, :, 0].astype(np.int64)
    cand_ops = np.zeros((B, NJ), np.int64)
    for bb in range(B):
        u = np.unique(cand0[bb])
        if len(u) >= NJ:
            cand_ops[bb] = u[:NJ]
        else:
            cand_ops[bb, : len(u)] = u
    # one-hot gather matrix, graph_pool packed as column 100
    ST = np.zeros((B, N, SC), np.float32)
    bidx = np.repeat(np.arange(B), NJ)
    ST[bidx, cand_ops.reshape(-1), np.tile(np.arange(NJ), B)] = 1.0
    ST[:, :, 100] = gpool

    ba3 = float(np.asarray(actor[2][1]).reshape(-1)[0])
    maskneg = np.where(mask, np.float32(NEG), np.float32(0.0)).astype(np.float32) + ba3

    shared = {
        "eps": eps.reshape(2, 1),
        "w1a": gin_params[0][0][0].astype(np.float32),
        "b1a": np.ascontiguousarray(
            gin_params[0][0][1].astype(np.float32).reshape(2, 128).T
        ),
        "w1b": gin_params[0][1][0].astype(np.float32).reshape(2, 128, H),
        "b1b": np.ascontiguousarray(
            gin_params[0][1][1].astype(np.float32).reshape(2, 128).T
        ),
        "w2a": gin_params[1][0][0].astype(np.float32).reshape(2, 128, H),
        "b2a": np.ascontiguousarray(
            gin_params[1][0][1].astype(np.float32).reshape(2, 128).T
        ),
        "w2b": gin_params[1][1][0].astype(np.float32).reshape(2, 128, H),
        "b2b": np.ascontiguousarray(
            gin_params[1][1][1].astype(np.float32).reshape(2, 128).T
        ),
        "wa1c": np.ascontiguousarray(actor[0][0][0:256]).astype(_nbf16).reshape(2, 128, HA),
        "wa1m": np.ascontiguousarray(actor[0][0][256:260]).astype(_nbf16),
        "wa1p": np.ascontiguousarray(actor[0][0][260:516]).astype(np.float32).reshape(2, 128, HA),
        "ba1": actor[0][1].astype(np.float32).reshape(HA, 1),
        "wa2": actor[1][0].astype(_nbf16),
        "ba2": actor[1][1].astype(np.float32).reshape(HA, 1),
        "wa3": actor[2][0].astype(_nbf16),
        "wc1": critic[0][0].astype(np.float32).reshape(2, 128, HA),
        "bc1": critic[0][1].astype(np.float32).reshape(HA, 1),
        "wc2": critic[1][0].astype(np.float32),
        "bc2": critic[1][1].astype(np.float32).reshape(HA, 1),
        "wc3": critic[2][0].astype(np.float32),
        "bc3": critic[2][1].astype(np.float32).reshape(1, 1),
    }

    adj_bf = adj.astype(_nbf16)
    x_bf = x.astype(_nbf16)
    mach_bf = mach.astype(_nbf16)
    ST_bf = ST.astype(_nbf16)

    in_maps = []
    for i in range(N_CORES):
        sl = slice(i * BPC, (i + 1) * BPC)
        m = dict(shared)
        m["adjT"] = np.ascontiguousarray(adj_bf[sl].transpose(0, 2, 1))
        m["x"] = np.ascontiguousarray(x_bf[sl])
        m["xT"] = np.ascontiguousarray(x_bf[sl].transpose(0, 2, 1))
        m["ST"] = np.ascontiguousarray(ST_bf[sl])
        m["machT"] = np.ascontiguousarray(mach_bf[sl].transpose(0, 2, 1))
        m["maskneg"] = np.ascontiguousarray(maskneg[sl].reshape(BPC, 1, N))
        in_maps.append(m)
    return in_maps


def _run(inputs, trace=False):
    in_maps = _prep_inputs(inputs)
    nc = _get_nc()
    res = run_bass_kernel_spmd(
        nc, in_maps, core_ids=list(range(N_CORES)), trace=trace
    )
    outs = np.concatenate([np.asarray(res.results[i]["out"]) for i in range(N_CORES)], axis=0)
    pi = outs[:, 0:1000].reshape(B, N, 1).astype(np.float32)
    v = outs[:, 1000:1001].astype(np.float32)
    return pi, v, res.exec_time_ns


def kernel(**inputs):
    pi, v, _ = _run(inputs, trace=False)
    return pi, v


# revision 26
# speedup vs baseline: 1.0113x; 1.0113x over previous
"""Trainium2 Bass kernel for the GIN ActorCritic forward pass.

Shards batch-parallel over 8 NeuronCores (4 graphs each). Host-side
preprocessing: transpose+bf16-cast adjacency, build one-hot candidate
gather matrix (torch.unique semantics) with graph_pool packed as an
extra column, fold actor bias b3 + mask into an additive score mask.
"""
import sys
import types

sys.path.insert(0, "/opt/trn_rl_repo")

import numpy as np
import ml_dtypes

import concourse.bass as bass
import concourse.mybir as mybir
import concourse.tile as tile
from concourse.vector_clock import ScopedClock
from concourse.masks import make_identity
from concourse.bass_utils import run_bass_kernel_spmd

BF16 = mybir.dt.bfloat16
F32 = mybir.dt.float32
F32R = mybir.dt.float32r
AF = mybir.ActivationFunctionType
ALU = mybir.AluOpType

B, N, D, H, HA = 32, 1000, 8, 256, 64
NJ, NM = 100, 10
N_CORES = 8
BPC = B // N_CORES  # 4 graphs per core
SC = 104  # ST columns: 100 one-hot cand cols + col 100 = graph_pool + pad
NT = 8  # node tiles of 128 (last is 104)
TS = [128] * 7 + [104]
CH = [(0, 512), (512, 488)]  # free-dim chunks for GIN stages
CHA = [(0, 500), (500, 500)]  # actor chunks (aligned to cand groups of 10)
NEG = -1.0e30

_nbf16 = ml_dtypes.bfloat16


# ---------------------------------------------------------------------------
# Tile drain patch: walrus in this image rejects >2 sync waits on a CTRL
# drain; split the final global-clock drain into one-wait-per-drain chain.
def _patched_drain_and_barrier(self, tick_clock, wait_clock):
    nc = self.nc
    drain_inst = nc.sync.drain()
    wait_clock.add_sem_waits(
        drain_inst.ins, ScopedClock({None: tick_clock.global_clock})
    )
    waits = list(drain_inst.ins.sync_info.on_wait or [])
    if len(waits) > 1:
        drain_inst.ins.sync_info.on_wait = waits[:1]
        for w in waits[1:]:
            d = nc.sync.drain()
            d.ins.sync_info = mybir.SyncInfo(on_wait=[w], on_update=[])
    nc.all_engine_barrier()
    popped = nc._tile_sem_poison_stack.pop()
    assert popped is self._sem_poison
    nc.clear_and_free_semaphores(list(self.sems.allocated().values()))
    nc.all_engine_barrier()


tile.TileContext._drain_and_barrier = _patched_drain_and_barrier

MAX_WAITS = 1


def _split_sync_waits(nc, max_waits=MAX_WAITS):
    """walrus in this image encodes at most `max_waits` sem-waits per
    instruction; hoist the excess into same-engine NoOps placed just
    before the instruction."""
    n_split = 0
    for f in nc.m.functions:
        for bb in f.blocks:
            insts = list(bb.instructions)
            out = []
            changed = False
            for inst in insts:
                si = inst.sync_info
                waits = list(si.on_wait) if (si is not None and si.on_wait) else []
                if len(waits) > max_waits:
                    changed = True
                    extra = waits[: len(waits) - max_waits]
                    for i in range(0, len(extra), max_waits):
                        chunk = extra[i : i + max_waits]
                        nop = mybir.InstNoOp(
                            name=f"I-wsplit-{n_split}",
                            engine=inst.engine,
                            ins=[],
                            outs=[],
                            sync_info=mybir.SyncInfo(on_wait=chunk, on_update=[]),
                        )
                        n_split += 1
                        out.append(nop)
                    si.on_wait = waits[len(waits) - max_waits :]
                out.append(inst)
            if changed:
                bb.instructions = out
    return n_split


def _install_ntff_shim():
    """Provide the missing antenv.axon_hooks so trace=True works (test.py)."""
    if "antenv.axon_hooks" in sys.modules:
        return
    mod = types.ModuleType("antenv.axon_hooks")
    mod._hook = None
    mod.set_axon_ntff_profile_hook = lambda h: setattr(mod, "_hook", h)
    mod.get_axon_ntff_profile_hook = lambda: mod._hook
    sys.modules["antenv.axon_hooks"] = mod
    import antenv

    antenv.axon_hooks = mod
    try:
        from trn_agent_boot.trn_boot import _ntff_profile_via_ctypes

        mod.set_axon_ntff_profile_hook(
            _ntff_profile_via_ctypes("/opt/axon/libaxon_pjrt.so")
        )
    except Exception:
        pass


def _bcast_ap(ap, count=128):
    """Partition-broadcast a [1,1]-style dram element to `count` partitions."""
    return bass.AP(tensor=ap.tensor, offset=ap.offset, ap=[[0, count]] + list(ap.ap))


def _rep10_ap(ap):
    """Append an inner stride-0 dim of 10 (repeat_interleave along free)."""
    return bass.AP(
        tensor=ap.tensor, offset=ap.offset, ap=list(ap.ap) + [[0, NM]]
    )


def _build_nc():
    nc = bass.Bass()

    # --- per-core sharded inputs -----------------------------------------
    adjT_e = nc.declare_dram_parameter("adjT", [BPC, N, N], BF16, isOutput=False)
    x_e = nc.declare_dram_parameter("x", [BPC, N, D], BF16, isOutput=False)
    xT_e = nc.declare_dram_parameter("xT", [BPC, D, N], BF16, isOutput=False)
    ST_e = nc.declare_dram_parameter("ST", [BPC, N, SC], BF16, isOutput=False)
    machT_e = nc.declare_dram_parameter("machT", [BPC, 4, N], BF16, isOutput=False)
    mneg_e = nc.declare_dram_parameter("maskneg", [BPC, 1, N], F32, isOutput=False)
    # --- replicated weights ----------------------------------------------
    eps_e = nc.declare_dram_parameter("eps", [2, 1], F32, isOutput=False)
    w1a_e = nc.declare_dram_parameter("w1a", [D, H], F32R, isOutput=False)
    w1b_e = nc.declare_dram_parameter("w1b", [2, 128, H], F32R, isOutput=False)
    w2a_e = nc.declare_dram_parameter("w2a", [2, 128, H], F32R, isOutput=False)
    w2b_e = nc.declare_dram_parameter("w2b", [2, 128, H], F32R, isOutput=False)
    b1a_e = nc.declare_dram_parameter("b1a", [128, 2], F32, isOutput=False)
    b1b_e = nc.declare_dram_parameter("b1b", [128, 2], F32, isOutput=False)
    b2a_e = nc.declare_dram_parameter("b2a", [128, 2], F32, isOutput=False)
    b2b_e = nc.declare_dram_parameter("b2b", [128, 2], F32, isOutput=False)
    wa1c_e = nc.declare_dram_parameter("wa1c", [2, 128, HA], BF16, isOutput=False)
    wa1p_e = nc.declare_dram_parameter("wa1p", [2, 128, HA], F32, isOutput=False)
    wa1m_e = nc.declare_dram_parameter("wa1m", [4, HA], BF16, isOutput=False)
    ba1_e = nc.declare_dram_parameter("ba1", [HA, 1], F32, isOutput=False)
    wa2_e = nc.declare_dram_parameter("wa2", [HA, HA], BF16, isOutput=False)
    ba2_e = nc.declare_dram_parameter("ba2", [HA, 1], F32, isOutput=False)
    wa3_e = nc.declare_dram_parameter("wa3", [HA, 1], BF16, isOutput=False)
    wc1_e = nc.declare_dram_parameter("wc1", [2, 128, HA], F32, isOutput=False)
    bc1_e = nc.declare_dram_parameter("bc1", [HA, 1], F32, isOutput=False)
    wc2_e = nc.declare_dram_parameter("wc2", [HA, HA], F32, isOutput=False)
    bc2_e = nc.declare_dram_parameter("bc2", [HA, 1], F32, isOutput=False)
    wc3_e = nc.declare_dram_parameter("wc3", [HA, 1], F32, isOutput=False)
    bc3_e = nc.declare_dram_parameter("bc3", [1, 1], F32, isOutput=False)
    out_e = nc.declare_dram_parameter("out", [BPC, 1001], F32, isOutput=True)

    from contextlib import ExitStack

    with tile.TileContext(nc) as tc, ExitStack() as ctx:
        wp = ctx.enter_context(tc.tile_pool(name="wp", bufs=1))
        ap_ = ctx.enter_context(tc.tile_pool(name="adj", bufs=2))
        sp = ctx.enter_context(tc.tile_pool(name="small", bufs=2))
        hp = ctx.enter_context(tc.tile_pool(name="acts", bufs=2))
        pmm = ctx.enter_context(tc.tile_pool(name="pmm", bufs=4, space="PSUM"))
        ptp = ctx.enter_context(tc.tile_pool(name="ptp", bufs=2, space="PSUM"))
        psm = ctx.enter_context(tc.tile_pool(name="psm", bufs=1, space="PSUM"))

        # ---- constants & weights (loaded once) --------------------------
        ident = wp.tile([128, 128], BF16)
        make_identity(nc, ident[:, :])

        eps0 = wp.tile([128, 1], F32, tag="eps0")
        eps1 = wp.tile([128, 1], F32, tag="eps1")
        e_ap = eps_e[:, :]
        nc.sync.dma_start(
            out=eps0[:, :],
            in_=bass.AP(tensor=e_ap.tensor, offset=e_ap.offset, ap=[[0, 128], [1, 1]]),
        )
        nc.sync.dma_start(
            out=eps1[:, :],
            in_=bass.AP(
                tensor=e_ap.tensor, offset=e_ap.offset + 1, ap=[[0, 128], [1, 1]]
            ),
        )
        # 1 + eps
        nc.scalar.add(out=eps0[:, :], in_=eps0[:, :], add=1.0)
        nc.scalar.add(out=eps1[:, :], in_=eps1[:, :], add=1.0)

        w1a = wp.tile([D, H], F32R, tag="w1a")
        nc.sync.dma_start(out=w1a[:, :], in_=w1a_e[:, :])
        gin_w = {}
        for nm, ext in (("w1b", w1b_e), ("w2a", w2a_e), ("w2b", w2b_e)):
            t = wp.tile([128, 2, H], F32R, tag=nm)
            for k in range(2):
                nc.sync.dma_start(out=t[:, k, :], in_=ext[k])
            gin_w[nm] = t
        gin_b = {}
        for nm, ext in (
            ("b1a", b1a_e),
            ("b1b", b1b_e),
            ("b2a", b2a_e),
            ("b2b", b2b_e),
        ):
            t = wp.tile([128, 2], F32, tag=nm)
            nc.sync.dma_start(out=t[:, :], in_=ext[:, :])
            gin_b[nm] = t
        wa1c = wp.tile([128, 2, HA], BF16, tag="wa1c")
        wa1p = wp.tile([128, 2, HA], F32, tag="wa1p")
        wc1 = wp.tile([128, 2, HA], F32, tag="wc1")
        for t, ext in ((wa1c, wa1c_e), (wa1p, wa1p_e), (wc1, wc1_e)):
            for k in range(2):
                nc.sync.dma_start(out=t[:, k, :], in_=ext[k])
        wa1m = wp.tile([4, HA], BF16, tag="wa1m")
        nc.sync.dma_start(out=wa1m[:, :], in_=wa1m_e[:, :])
        wa2 = wp.tile([HA, HA], BF16, tag="wa2")
        nc.sync.dma_start(out=wa2[:, :], in_=wa2_e[:, :])
        wa3 = wp.tile([HA, 1], BF16, tag="wa3")
        nc.sync.dma_start(out=wa3[:, :], in_=wa3_e[:, :])
        wc2 = wp.tile([HA, HA], F32, tag="wc2")
        nc.sync.dma_start(out=wc2[:, :], in_=wc2_e[:, :])
        wc3 = wp.tile([HA, 1], F32, tag="wc3")
        nc.sync.dma_start(out=wc3[:, :], in_=wc3_e[:, :])
        ba1 = wp.tile([HA, 1], F32, tag="ba1")
        nc.sync.dma_start(out=ba1[:, :], in_=ba1_e[:, :])
        ba2 = wp.tile([HA, 1], F32, tag="ba2")
        nc.sync.dma_start(out=ba2[:, :], in_=ba2_e[:, :])
        bc1 = wp.tile([HA, 1], F32, tag="bc1")
        nc.sync.dma_start(out=bc1[:, :], in_=bc1_e[:, :])
        bc2 = wp.tile([HA, 1], F32, tag="bc2")
        nc.sync.dma_start(out=bc2[:, :], in_=bc2_e[:, :])
        bc3 = wp.tile([1, 1], F32, tag="bc3")
        nc.sync.dma_start(out=bc3[:, :], in_=bc3_e[:, :])

        for b in range(BPC):
            # ---- load per-batch inputs ----------------------------------
            adjT = ap_.tile([128, 7, N], BF16, tag="adjT")
            adjTt = ap_.tile([128, N], BF16, tag="adjTt")
            nc.sync.dma_start(
                out=adjT[:, :, :],
                in_=adjT_e[b, 0:896, :].rearrange("(j p) i -> p j i", p=128),
            )
            nc.sync.dma_start(out=adjTt[0:104, :], in_=adjT_e[b, 896:1000, :])

            x_sb = sp.tile([128, 7, D], BF16, tag="x")
            x_tl = sp.tile([128, D], BF16, tag="xt")
            nc.sync.dma_start(
                out=x_sb[:, :, :],
                in_=x_e[b, 0:896, :].rearrange("(j p) d -> p j d", p=128),
            )
            nc.sync.dma_start(out=x_tl[0:104, :], in_=x_e[b, 896:1000, :])

            ST_sb = sp.tile([128, 7, SC], BF16, tag="ST")
            ST_tl = sp.tile([128, SC], BF16, tag="STt")
            nc.sync.dma_start(
                out=ST_sb[:, :, :],
                in_=ST_e[b, 0:896, :].rearrange("(j p) c -> p j c", p=128),
            )
            nc.sync.dma_start(out=ST_tl[0:104, :], in_=ST_e[b, 896:1000, :])

            xT_sb = sp.tile([D, N], BF16, tag="xT")
            nc.sync.dma_start(out=xT_sb[:, :], in_=xT_e[b])
            machT = sp.tile([4, N], BF16, tag="machT")
            nc.sync.dma_start(out=machT[:, :], in_=machT_e[b])
            mneg = sp.tile([1, N], F32, tag="mneg")
            nc.sync.dma_start(out=mneg[:, :], in_=mneg_e[b])

            def adjT_blk(j, c0, cn):
                if j < 7:
                    return adjT[:, j, c0 : c0 + cn]
                return adjTt[0:104, c0 : c0 + cn]

            def x_blk(j):
                if j < 7:
                    return x_sb[:, j, :]
                return x_tl[0:104, :]

            def ST_blk(j):
                if j < 7:
                    return ST_sb[:, j, 0:SC]
                return ST_tl[0:104, 0:SC]

            # ---- GIN layer 1 --------------------------------------------
            # pooled1T[d, i] = sum_j x[j, d] * adjT[j, i]  (+ (1+eps0)*xT)
            p1T = hp.tile([D, N], F32R, tag="p1T")
            for c0, cn in CH:
                q = pmm.tile([D, 512], F32, tag="mm")
                for j in range(NT):
                    nc.tensor.matmul(
                        q[0:D, 0:cn],
                        lhsT=x_blk(j),
                        rhs=adjT_blk(j, c0, cn),
                        start=(j == 0),
                        stop=(j == NT - 1),
                    )
                nc.vector.scalar_tensor_tensor(
                    out=p1T[:, c0 : c0 + cn],
                    in0=xT_sb[:, c0 : c0 + cn],
                    scalar=eps0[0:D, :],
                    in1=q[0:D, 0:cn],
                    op0=ALU.mult,
                    op1=ALU.add,
                )

            # relu1T = relu(w1a^T @ p1T + b1a)
            r1T = hp.tile([128, 2, N], F32R, tag="r1T")
            for m in range(2):
                for c0, cn in CH:
                    q = pmm.tile([128, 512], F32, tag="mm")
                    nc.tensor.matmul(
                        q[:, 0:cn],
                        lhsT=w1a[:, m * 128 : (m + 1) * 128],
                        rhs=p1T[:, c0 : c0 + cn],
                        start=True,
                        stop=True,
                    )
                    nc.scalar.activation(
                        out=r1T[:, m, c0 : c0 + cn],
                        in_=q[:, 0:cn],
                        func=AF.Relu,
                        bias=gin_b["b1a"][:, m : m + 1],
                    )

            # h1T = relu(w1b^T @ r1T + b1b); h1 node-major via PE transpose
            h1T = hp.tile([128, 2, N], BF16, tag="h1T")
            for m in range(2):
                for c0, cn in CH:
                    q = pmm.tile([128, 512], F32, tag="mm")
                    for k in range(2):
                        nc.tensor.matmul(
                            q[:, 0:cn],
                            lhsT=gin_w["w1b"][:, k, m * 128 : (m + 1) * 128],
                            rhs=r1T[:, k, c0 : c0 + cn],
                            start=(k == 0),
                            stop=(k == 1),
                        )
                    nc.scalar.activation(
                        out=h1T[:, m, c0 : c0 + cn],
                        in_=q[:, 0:cn],
                        func=AF.Relu,
                        bias=gin_b["b1b"][:, m : m + 1],
                    )
            h1nm = hp.tile([128, NT, H], BF16, tag="h1nm")
            for m in range(2):
                for j in range(NT):
                    tsz = TS[j]
                    tq = ptp.tile([128, 128], BF16, tag="tp")
                    nc.tensor.transpose(
                        tq[0:tsz, 0:128],
                        in_=h1T[:, m, j * 128 : j * 128 + tsz],
                        identity=ident[:, :],
                    )
                    nc.vector.tensor_copy(
                        out=h1nm[0:tsz, j, m * 128 : (m + 1) * 128],
                        in_=tq[0:tsz, 0:128],
                    )

            # ---- GIN layer 2 --------------------------------------------
            p2T = hp.tile([128, 2, N], F32R, tag="p2T")
            for m in range(2):
                for c0, cn in CH:
                    q = pmm.tile([128, 512], F32, tag="mm")
                    for j in range(NT):
                        nc.tensor.matmul(
                            q[:, 0:cn],
                            lhsT=h1nm[0 : TS[j], j, m * 128 : (m + 1) * 128],
                            rhs=adjT_blk(j, c0, cn),
                            start=(j == 0),
                            stop=(j == NT - 1),
                        )
                    nc.vector.scalar_tensor_tensor(
                        out=p2T[:, m, c0 : c0 + cn],
                        in0=h1T[:, m, c0 : c0 + cn],
                        scalar=eps1[:, :],
                        in1=q[:, 0:cn],
                        op0=ALU.mult,
                        op1=ALU.add,
                    )

            r2T = hp.tile([128, 2, N], F32R, tag="r2T")
            for m in range(2):
                for c0, cn in CH:
                    q = pmm.tile([128, 512], F32, tag="mm")
                    for k in range(2):
                        nc.tensor.matmul(
                            q[:, 0:cn],
                            lhsT=gin_w["w2a"][:, k, m * 128 : (m + 1) * 128],
                            rhs=p2T[:, k, c0 : c0 + cn],
                            start=(k == 0),
                            stop=(k == 1),
                        )
                    nc.scalar.activation(
                        out=r2T[:, m, c0 : c0 + cn],
                        in_=q[:, 0:cn],
                        func=AF.Relu,
                        bias=gin_b["b2a"][:, m : m + 1],
                    )

            h2T = hp.tile([128, 2, N], BF16, tag="h2T")
            for m in range(2):
                for c0, cn in CH:
                    q = pmm.tile([128, 512], F32, tag="mm")
                    for k in range(2):
                        nc.tensor.matmul(
                            q[:, 0:cn],
                            lhsT=gin_w["w2b"][:, k, m * 128 : (m + 1) * 128],
                            rhs=r2T[:, k, c0 : c0 + cn],
                            start=(k == 0),
                            stop=(k == 1),
                        )
                    nc.scalar.activation(
                        out=h2T[:, m, c0 : c0 + cn],
                        in_=q[:, 0:cn],
                        func=AF.Relu,
                        bias=gin_b["b2b"][:, m : m + 1],
                    )
            h2nm = hp.tile([128, NT, H], BF16, tag="h2nm")
            for m in range(2):
                for j in range(NT):
                    tsz = TS[j]
                    tq = ptp.tile([128, 128], BF16, tag="tp")
                    nc.tensor.transpose(
                        tq[0:tsz, 0:128],
                        in_=h2T[:, m, j * 128 : j * 128 + tsz],
                        identity=ident[:, :],
                    )
                    nc.vector.tensor_copy(
                        out=h2nm[0:tsz, j, m * 128 : (m + 1) * 128],
                        in_=tq[0:tsz, 0:128],
                    )

            # ---- candidate gather + graph pool (one matmul) --------------
            # cfT[d, c] = sum_n h2[n, d] * ST[n, c]; col 100 = h_pooled
            cfT = hp.tile([128, 2, SC], BF16, tag="cfT")
            hp32 = hp.tile([128, 2], F32, tag="hp32")
            for m in range(2):
                q = ptp.tile([128, SC], F32, tag="tp")
                for j in range(NT):
                    nc.tensor.matmul(
                        q[:, 0:SC],
                        lhsT=h2nm[0 : TS[j], j, m * 128 : (m + 1) * 128],
                        rhs=ST_blk(j),
                        start=(j == 0),
                        stop=(j == NT - 1),
                    )
                nc.scalar.copy(out=cfT[:, m, :], in_=q[:, 0:SC])
                nc.scalar.copy(out=hp32[:, m : m + 1], in_=q[:, 100:101])

            # ---- actor bias u = wa1p^T @ h_pooled + ba1 ------------------
            qu = psm.tile([HA, 1], F32, tag="qu")
            for k in range(2):
                nc.tensor.matmul(
                    qu[:, :],
                    lhsT=wa1p[:, k, :],
                    rhs=hp32[:, k : k + 1],
                    start=(k == 0),
                    stop=(k == 1),
                )
            ua = hp.tile([HA, 1], F32, tag="ua")
            nc.vector.tensor_add(out=ua[:, :], in0=qu[:, :], in1=ba1[:, :])

            # ---- actor layer 1: cand(rep10) + mach + bias ----------------
            a1T = hp.tile([HA, N], BF16, tag="a1T")
            for c0, cn in CHA:
                q = pmm.tile([HA, 512], F32, tag="mm")
                for k in range(2):
                    src = cfT[:, k, c0 // NM : (c0 + cn) // NM]
                    nc.tensor.matmul(
                        q[:, 0:cn],
                        lhsT=wa1c[:, k, :],
                        rhs=_rep10_ap(src),
                        start=(k == 0),
                        stop=False,
                    )
                nc.tensor.matmul(
                    q[:, 0:cn],
                    lhsT=wa1m[:, :],
                    rhs=machT[:, c0 : c0 + cn],
                    start=False,
                    stop=True,
                )
                nc.scalar.activation(
                    out=a1T[:, c0 : c0 + cn],
                    in_=q[:, 0:cn],
                    func=AF.Tanh,
                    bias=ua[:, :],
                )

            # ---- actor layer 2 ------------------------------------------
            a2T = hp.tile([HA, N], BF16, tag="a2T")
            for c0, cn in CHA:
                q = pmm.tile([HA, 512], F32, tag="mm")
                nc.tensor.matmul(
                    q[:, 0:cn],
                    lhsT=wa2[:, :],
                    rhs=a1T[:, c0 : c0 + cn],
                    start=True,
                    stop=True,
                )
                nc.scalar.activation(
                    out=a2T[:, c0 : c0 + cn],
                    in_=q[:, 0:cn],
                    func=AF.Tanh,
                    bias=ba2[:, :],
                )

            # ---- scores + mask (+ba3 folded into maskneg) ----------------
            sT = hp.tile([1, N], F32, tag="sT")
            for c0, cn in CHA:
                q = pmm.tile([1, 512], F32, tag="mm")
                nc.tensor.matmul(
                    q[0:1, 0:cn],
                    lhsT=wa3[:, :],
                    rhs=a2T[:, c0 : c0 + cn],
                    start=True,
                    stop=True,
                )
                nc.vector.tensor_add(
                    out=sT[:, c0 : c0 + cn],
                    in0=q[0:1, 0:cn],
                    in1=mneg[:, c0 : c0 + cn],
                )

            # ---- masked softmax over the 1000 candidates -----------------
            nmx = hp.tile([1, 1], F32, tag="nmx")
            nc.vector.reduce_max(out=nmx[:, :], in_=sT[:, :], axis=mybir.AxisListType.X, negate=True)
            esb = hp.tile([1, N], F32, tag="esb")
            ssum = hp.tile([1, 1], F32, tag="ssum")
            nc.scalar.activation(
                out=esb[:, :],
                in_=sT[:, :],
                func=AF.Exp,
                bias=nmx[:, :],
                accum_out=ssum[:, :],
            )
            rsum = hp.tile([1, 1], F32, tag="rsum")
            nc.vector.reciprocal(out=rsum[:, :], in_=ssum[:, :])
            pi = hp.tile([1, N], F32, tag="pi")
            nc.vector.tensor_scalar_mul(pi[:, :], in0=esb[:, :], scalar1=rsum[:, :])
            nc.sync.dma_start(out=out_e[b : b + 1, 0:1000], in_=pi[:, :])

            # ---- critic head --------------------------------------------
            qc1 = psm.tile([HA, 1], F32, tag="qu")
            for k in range(2):
                nc.tensor.matmul(
                    qc1[:, :],
                    lhsT=wc1[:, k, :],
                    rhs=hp32[:, k : k + 1],
                    start=(k == 0),
                    stop=(k == 1),
                )
            c1 = hp.tile([HA, 1], F32, tag="c1")
            nc.scalar.activation(out=c1[:, :], in_=qc1[:, :], func=AF.Tanh, bias=bc1[:, :])
            qc2 = psm.tile([HA, 1], F32, tag="qu")
            nc.tensor.matmul(qc2[:, :], lhsT=wc2[:, :], rhs=c1[:, :], start=True, stop=True)
            c2 = hp.tile([HA, 1], F32, tag="c2")
            nc.scalar.activation(out=c2[:, :], in_=qc2[:, :], func=AF.Tanh, bias=bc2[:, :])
            qv = psm.tile([1, 1], F32, tag="qu")
            nc.tensor.matmul(qv[:, :], lhsT=wc3[:, :], rhs=c2[:, :], start=True, stop=True)
            v = hp.tile([1, 1], F32, tag="v")
            nc.scalar.activation(out=v[:, :], in_=qv[:, :], func=AF.Identity, bias=bc3[:, :])
            nc.sync.dma_start(out=out_e[b : b + 1, 1000:1001], in_=v[:, :])

    _split_sync_waits(nc)
    return nc


_NC_CACHE = {}


def _get_nc():
    if "nc" not in _NC_CACHE:
        _NC_CACHE["nc"] = _build_nc()
    return _NC_CACHE["nc"]


def _leaf(a):
    return np.asarray(a)


def _prep_inputs(inputs):
    x = _leaf(inputs["x"]).astype(np.float32)
    adj = _leaf(inputs["adj_matrix"]).astype(np.float32)
    gpool = _leaf(inputs["graph_pool"]).astype(np.float32)
    cand = _leaf(inputs["candidate"])
    mask = _leaf(inputs["mask"])
    mach = _leaf(inputs["machine_feat"]).astype(np.float32)
    gin_params = [[(_leaf(w), _leaf(bb)) for (w, bb) in layer] for layer in inputs["gin_params"]]
    eps = _leaf(inputs["eps"]).astype(np.float32)
    actor = [(_leaf(w), _leaf(bb)) for (w, bb) in inputs["actor_params"]]
    critic = [(_leaf(w), _leaf(bb)) for (w, bb) in inputs["critic_params"]]

    # torch.unique semantics (jnp.unique size=NJ fill=0): sorted unique,
    # truncated/padded to NJ
    cand0 = cand[:, :, 0].astype(np.int64)
    cand_ops = np.zeros((B, NJ), np.int64)
    for bb in range(B):
        u = np.unique(cand0[bb])
        if len(u) >= NJ:
            cand_ops[bb] = u[:NJ]
        else:
            cand_ops[bb, : len(u)] = u
    # one-hot gather matrix, graph_pool packed as column 100
    ST = np.zeros((B, N, SC), np.float32)
    bidx = np.repeat(np.arange(B), NJ)
    ST[bidx, cand_ops.reshape(-1), np.tile(np.arange(NJ), B)] = 1.0
    ST[:, :, 100] = gpool

    ba3 = float(np.asarray(actor[2][1]).reshape(-1)[0])
    maskneg = np.where(mask, np.float32(NEG), np.float32(0.0)).astype(np.float32) + ba3

    shared = {
        "eps": eps.reshape(2, 1),
        "w1a": gin_params[0][0][0].astype(np.float32),
        "b1a": np.ascontiguousarray(
            gin_params[0][0][1].astype(np.float32).reshape(2, 128).T
        ),
        "w1b": gin_params[0][1][0].astype(np.float32).reshape(2, 128, H),
        "b1b": np.ascontiguousarray(
            gin_params[0][1][1].astype(np.float32).reshape(2, 128).T
        ),
        "w2a": gin_params[1][0][0].astype(np.float32).reshape(2, 128, H),
        "b2a": np.ascontiguousarray(
            gin_params[1][0][1].astype(np.float32).reshape(2, 128).T
        ),
        "w2b": gin_params[1][1][0].astype(np.float32).reshape(2, 128, H),
        "b2b": np.ascontiguousarray(
            gin_params[1][1][1].astype(np.float32).reshape(2, 128).T
        ),
        "wa1c": np.ascontiguousarray(actor[0][0][0:256]).astype(_nbf16).reshape(2, 128, HA),
        "wa1m": np.ascontiguousarray(actor[0][0][256:260]).astype(_nbf16),
        "wa1p": np.ascontiguousarray(actor[0][0][260:516]).astype(np.float32).reshape(2, 128, HA),
        "ba1": actor[0][1].astype(np.float32).reshape(HA, 1),
        "wa2": actor[1][0].astype(_nbf16),
        "ba2": actor[1][1].astype(np.float32).reshape(HA, 1),
        "wa3": actor[2][0].astype(_nbf16),
        "wc1": critic[0][0].astype(np.float32).reshape(2, 128, HA),
        "bc1": critic[0][1].astype(np.float32).reshape(HA, 1),
        "wc2": critic[1][0].astype(np.float32),
        "bc2": critic[1][1].astype(np.float32).reshape(HA, 1),
        "wc3": critic[2][0].astype(np.float32),
        "bc3": critic[2][1].astype(np.float32).reshape(1, 1),
    }

    adj_bf = adj.astype(_nbf16)
    x_bf = x.astype(_nbf16)
    mach_bf = mach.astype(_nbf16)
    ST_bf = ST.astype(_nbf16)

    in_maps = []
    for i in range(N_CORES):
        sl = slice(i * BPC, (i + 1) * BPC)
        m = dict(shared)
        m["adjT"] = np.ascontiguousarray(adj_bf[sl].transpose(0, 2, 1))
        m["x"] = np.ascontiguousarray(x_bf[sl])
        m["xT"] = np.ascontiguousarray(x_bf[sl].transpose(0, 2, 1))
        m["ST"] = np.ascontiguousarray(ST_bf[sl])
        m["machT"] = np.ascontiguousarray(mach_bf[sl].transpose(0, 2, 1))
        m["maskneg"] = np.ascontiguousarray(maskneg[sl].reshape(BPC, 1, N))
        in_maps.append(m)
    return in_maps


def _run(inputs, trace=False):
    in_maps = _prep_inputs(inputs)
    nc = _get_nc()
    res = run_bass_kernel_spmd(
        nc, in_maps, core_ids=list(range(N_CORES)), trace=trace
    )
    outs = np.concatenate([np.asarray(res.results[i]["out"]) for i in range(N_CORES)], axis=0)
    pi = outs[:, 0:1000].reshape(B, N, 1).astype(np.float32)
    v = outs[:, 1000:1001].astype(np.float32)
    return pi, v, res.exec_time_ns


def kernel(**inputs):
    pi, v, _ = _run(inputs, trace=False)
    return pi, v


# revision 27
# speedup vs baseline: 1.1737x; 1.1606x over previous
"""Trainium2 Bass kernel for the GIN ActorCritic forward pass.

Shards batch-parallel over 8 NeuronCores (4 graphs each). Host-side
preprocessing: transpose+bf16-cast adjacency, build one-hot candidate
gather matrix (torch.unique semantics) with graph_pool packed as an
extra column, fold actor bias b3 + mask into an additive score mask.
"""
import sys
import types

sys.path.insert(0, "/opt/trn_rl_repo")

import numpy as np
import ml_dtypes

import concourse.bass as bass
import concourse.mybir as mybir
import concourse.tile as tile
from concourse.vector_clock import ScopedClock
from concourse.masks import make_identity
from concourse.bass_utils import run_bass_kernel_spmd

BF16 = mybir.dt.bfloat16
F32 = mybir.dt.float32
F32R = mybir.dt.float32r
AF = mybir.ActivationFunctionType
ALU = mybir.AluOpType

B, N, D, H, HA = 32, 1000, 8, 256, 64
NJ, NM = 100, 10
N_CORES = 8
BPC = B // N_CORES  # 4 graphs per core
SC = 104  # ST columns: 100 one-hot cand cols + col 100 = graph_pool + pad
NT = 8  # node tiles of 128 (last is 104)
TS = [128] * 7 + [104]
CH = [(0, 512), (512, 488)]  # free-dim chunks for GIN stages
CHA = [(0, 500), (500, 500)]  # actor chunks (aligned to cand groups of 10)
NEG = -1.0e30

_nbf16 = ml_dtypes.bfloat16


# ---------------------------------------------------------------------------
# Tile drain patch: walrus in this image rejects >2 sync waits on a CTRL
# drain; split the final global-clock drain into one-wait-per-drain chain.
def _patched_drain_and_barrier(self, tick_clock, wait_clock):
    nc = self.nc
    drain_inst = nc.sync.drain()
    wait_clock.add_sem_waits(
        drain_inst.ins, ScopedClock({None: tick_clock.global_clock})
    )
    waits = list(drain_inst.ins.sync_info.on_wait or [])
    if len(waits) > 1:
        drain_inst.ins.sync_info.on_wait = waits[:1]
        for w in waits[1:]:
            d = nc.sync.drain()
            d.ins.sync_info = mybir.SyncInfo(on_wait=[w], on_update=[])
    nc.all_engine_barrier()
    popped = nc._tile_sem_poison_stack.pop()
    assert popped is self._sem_poison
    nc.clear_and_free_semaphores(list(self.sems.allocated().values()))
    nc.all_engine_barrier()


tile.TileContext._drain_and_barrier = _patched_drain_and_barrier

MAX_WAITS = 1


def _split_sync_waits(nc, max_waits=MAX_WAITS):
    """walrus in this image encodes at most `max_waits` sem-waits per
    instruction; hoist the excess into same-engine NoOps placed just
    before the instruction."""
    n_split = 0
    for f in nc.m.functions:
        for bb in f.blocks:
            insts = list(bb.instructions)
            out = []
            changed = False
            for inst in insts:
                si = inst.sync_info
                waits = list(si.on_wait) if (si is not None and si.on_wait) else []
                if len(waits) > max_waits:
                    changed = True
                    extra = waits[: len(waits) - max_waits]
                    for i in range(0, len(extra), max_waits):
                        chunk = extra[i : i + max_waits]
                        nop = mybir.InstNoOp(
                            name=f"I-wsplit-{n_split}",
                            engine=inst.engine,
                            ins=[],
                            outs=[],
                            sync_info=mybir.SyncInfo(on_wait=chunk, on_update=[]),
                        )
                        n_split += 1
                        out.append(nop)
                    si.on_wait = waits[len(waits) - max_waits :]
                out.append(inst)
            if changed:
                bb.instructions = out
    return n_split


def _install_ntff_shim():
    """Provide the missing antenv.axon_hooks so trace=True works (test.py)."""
    if "antenv.axon_hooks" in sys.modules:
        return
    mod = types.ModuleType("antenv.axon_hooks")
    mod._hook = None
    mod.set_axon_ntff_profile_hook = lambda h: setattr(mod, "_hook", h)
    mod.get_axon_ntff_profile_hook = lambda: mod._hook
    sys.modules["antenv.axon_hooks"] = mod
    import antenv

    antenv.axon_hooks = mod
    try:
        from trn_agent_boot.trn_boot import _ntff_profile_via_ctypes

        mod.set_axon_ntff_profile_hook(
            _ntff_profile_via_ctypes("/opt/axon/libaxon_pjrt.so")
        )
    except Exception:
        pass


def _bcast_ap(ap, count=128):
    """Partition-broadcast a [1,1]-style dram element to `count` partitions."""
    return bass.AP(tensor=ap.tensor, offset=ap.offset, ap=[[0, count]] + list(ap.ap))


def _rep10_ap(ap):
    """Append an inner stride-0 dim of 10 (repeat_interleave along free)."""
    return bass.AP(
        tensor=ap.tensor, offset=ap.offset, ap=list(ap.ap) + [[0, NM]]
    )


def _build_nc():
    nc = bass.Bass()

    # --- per-core sharded inputs -----------------------------------------
    adjT_e = nc.declare_dram_parameter("adjT", [BPC, N, N], BF16, isOutput=False)
    x_e = nc.declare_dram_parameter("x", [BPC, N, D], BF16, isOutput=False)
    xT_e = nc.declare_dram_parameter("xT", [BPC, D, N], BF16, isOutput=False)
    ST_e = nc.declare_dram_parameter("ST", [BPC, N, SC], BF16, isOutput=False)
    machT_e = nc.declare_dram_parameter("machT", [BPC, 4, N], BF16, isOutput=False)
    mneg_e = nc.declare_dram_parameter("maskneg", [BPC, 1, N], F32, isOutput=False)
    # --- replicated weights ----------------------------------------------
    eps_e = nc.declare_dram_parameter("eps", [2, 1], F32, isOutput=False)
    w1a_e = nc.declare_dram_parameter("w1a", [D, H], F32R, isOutput=False)
    w1b_e = nc.declare_dram_parameter("w1b", [2, 128, H], F32R, isOutput=False)
    w2a_e = nc.declare_dram_parameter("w2a", [2, 128, H], F32R, isOutput=False)
    w2b_e = nc.declare_dram_parameter("w2b", [2, 128, H], F32R, isOutput=False)
    b1a_e = nc.declare_dram_parameter("b1a", [128, 2], F32, isOutput=False)
    b1b_e = nc.declare_dram_parameter("b1b", [128, 2], F32, isOutput=False)
    b2a_e = nc.declare_dram_parameter("b2a", [128, 2], F32, isOutput=False)
    b2b_e = nc.declare_dram_parameter("b2b", [128, 2], F32, isOutput=False)
    wa1c_e = nc.declare_dram_parameter("wa1c", [2, 128, HA], BF16, isOutput=False)
    wa1p_e = nc.declare_dram_parameter("wa1p", [2, 128, HA], F32, isOutput=False)
    wa1m_e = nc.declare_dram_parameter("wa1m", [4, HA], BF16, isOutput=False)
    ba1_e = nc.declare_dram_parameter("ba1", [HA, 1], F32, isOutput=False)
    wa2_e = nc.declare_dram_parameter("wa2", [HA, HA], BF16, isOutput=False)
    ba2_e = nc.declare_dram_parameter("ba2", [HA, 1], F32, isOutput=False)
    wa3_e = nc.declare_dram_parameter("wa3", [HA, 1], BF16, isOutput=False)
    wc1_e = nc.declare_dram_parameter("wc1", [2, 128, HA], F32, isOutput=False)
    bc1_e = nc.declare_dram_parameter("bc1", [HA, 1], F32, isOutput=False)
    wc2_e = nc.declare_dram_parameter("wc2", [HA, HA], F32, isOutput=False)
    bc2_e = nc.declare_dram_parameter("bc2", [HA, 1], F32, isOutput=False)
    wc3_e = nc.declare_dram_parameter("wc3", [HA, 1], F32, isOutput=False)
    bc3_e = nc.declare_dram_parameter("bc3", [1, 1], F32, isOutput=False)
    out_e = nc.declare_dram_parameter("out", [BPC, 1001], F32, isOutput=True)

    from contextlib import ExitStack

    with tile.TileContext(nc) as tc, ExitStack() as ctx:
        wp = ctx.enter_context(tc.tile_pool(name="wp", bufs=1))
        ap_ = ctx.enter_context(tc.tile_pool(name="adj", bufs=2))
        sp = ctx.enter_context(tc.tile_pool(name="small", bufs=2))
        hp = ctx.enter_context(tc.tile_pool(name="acts", bufs=2))
        pmm = ctx.enter_context(tc.tile_pool(name="pmm", bufs=3, space="PSUM"))
        ptp = ctx.enter_context(tc.tile_pool(name="ptp", bufs=2, space="PSUM"))
        psm = ctx.enter_context(tc.tile_pool(name="psm", bufs=2, space="PSUM"))

        # ---- constants & weights (loaded once) --------------------------
        ident = wp.tile([128, 128], BF16)
        make_identity(nc, ident[:, :])

        eps0 = wp.tile([128, 1], F32, tag="eps0")
        eps1 = wp.tile([128, 1], F32, tag="eps1")
        e_ap = eps_e[:, :]
        nc.sync.dma_start(
            out=eps0[:, :],
            in_=bass.AP(tensor=e_ap.tensor, offset=e_ap.offset, ap=[[0, 128], [1, 1]]),
        )
        nc.sync.dma_start(
            out=eps1[:, :],
            in_=bass.AP(
                tensor=e_ap.tensor, offset=e_ap.offset + 1, ap=[[0, 128], [1, 1]]
            ),
        )
        # 1 + eps
        nc.scalar.add(out=eps0[:, :], in_=eps0[:, :], add=1.0)
        nc.scalar.add(out=eps1[:, :], in_=eps1[:, :], add=1.0)

        w1a = wp.tile([D, H], F32R, tag="w1a")
        nc.sync.dma_start(out=w1a[:, :], in_=w1a_e[:, :])
        gin_w = {}
        for nm, ext in (("w1b", w1b_e), ("w2a", w2a_e), ("w2b", w2b_e)):
            t = wp.tile([128, 2, H], F32R, tag=nm)
            for k in range(2):
                nc.sync.dma_start(out=t[:, k, :], in_=ext[k])
            gin_w[nm] = t
        gin_b = {}
        for nm, ext in (
            ("b1a", b1a_e),
            ("b1b", b1b_e),
            ("b2a", b2a_e),
            ("b2b", b2b_e),
        ):
            t = wp.tile([128, 2], F32, tag=nm)
            nc.sync.dma_start(out=t[:, :], in_=ext[:, :])
            gin_b[nm] = t
        wa1c = wp.tile([128, 2, HA], BF16, tag="wa1c")
        wa1p = wp.tile([128, 2, HA], F32, tag="wa1p")
        wc1 = wp.tile([128, 2, HA], F32, tag="wc1")
        for t, ext in ((wa1c, wa1c_e), (wa1p, wa1p_e), (wc1, wc1_e)):
            for k in range(2):
                nc.sync.dma_start(out=t[:, k, :], in_=ext[k])
        wa1m = wp.tile([4, HA], BF16, tag="wa1m")
        nc.sync.dma_start(out=wa1m[:, :], in_=wa1m_e[:, :])
        wa2 = wp.tile([HA, HA], BF16, tag="wa2")
        nc.sync.dma_start(out=wa2[:, :], in_=wa2_e[:, :])
        wa3 = wp.tile([HA, 1], BF16, tag="wa3")
        nc.sync.dma_start(out=wa3[:, :], in_=wa3_e[:, :])
        wc2 = wp.tile([HA, HA], F32, tag="wc2")
        nc.sync.dma_start(out=wc2[:, :], in_=wc2_e[:, :])
        wc3 = wp.tile([HA, 1], F32, tag="wc3")
        nc.sync.dma_start(out=wc3[:, :], in_=wc3_e[:, :])
        ba1 = wp.tile([HA, 1], F32, tag="ba1")
        nc.sync.dma_start(out=ba1[:, :], in_=ba1_e[:, :])
        ba2 = wp.tile([HA, 1], F32, tag="ba2")
        nc.sync.dma_start(out=ba2[:, :], in_=ba2_e[:, :])
        bc1 = wp.tile([HA, 1], F32, tag="bc1")
        nc.sync.dma_start(out=bc1[:, :], in_=bc1_e[:, :])
        bc2 = wp.tile([HA, 1], F32, tag="bc2")
        nc.sync.dma_start(out=bc2[:, :], in_=bc2_e[:, :])
        bc3 = wp.tile([1, 1], F32, tag="bc3")
        nc.sync.dma_start(out=bc3[:, :], in_=bc3_e[:, :])

        for b in range(BPC):
            # ---- load per-batch inputs ----------------------------------
            adjT = ap_.tile([128, 7, N], BF16, tag="adjT")
            adjTt = ap_.tile([128, N], BF16, tag="adjTt")
            nc.sync.dma_start(
                out=adjT[:, :, :],
                in_=adjT_e[b, 0:896, :].rearrange("(j p) i -> p j i", p=128),
            )
            nc.sync.dma_start(out=adjTt[0:104, :], in_=adjT_e[b, 896:1000, :])

            x_sb = sp.tile([128, 7, D], BF16, tag="x")
            x_tl = sp.tile([128, D], BF16, tag="xt")
            nc.sync.dma_start(
                out=x_sb[:, :, :],
                in_=x_e[b, 0:896, :].rearrange("(j p) d -> p j d", p=128),
            )
            nc.sync.dma_start(out=x_tl[0:104, :], in_=x_e[b, 896:1000, :])

            ST_sb = sp.tile([128, 7, SC], BF16, tag="ST")
            ST_tl = sp.tile([128, SC], BF16, tag="STt")
            nc.sync.dma_start(
                out=ST_sb[:, :, :],
                in_=ST_e[b, 0:896, :].rearrange("(j p) c -> p j c", p=128),
            )
            nc.sync.dma_start(out=ST_tl[0:104, :], in_=ST_e[b, 896:1000, :])

            xT_sb = sp.tile([D, N], BF16, tag="xT")
            nc.sync.dma_start(out=xT_sb[:, :], in_=xT_e[b])
            machT = sp.tile([4, N], BF16, tag="machT")
            nc.sync.dma_start(out=machT[:, :], in_=machT_e[b])
            mneg = sp.tile([1, N], F32, tag="mneg")
            nc.sync.dma_start(out=mneg[:, :], in_=mneg_e[b])

            def adjT_blk(j, c0, cn):
                if j < 7:
                    return adjT[:, j, c0 : c0 + cn]
                return adjTt[0:104, c0 : c0 + cn]

            def x_blk(j):
                if j < 7:
                    return x_sb[:, j, :]
                return x_tl[0:104, :]

            def ST_blk(j):
                if j < 7:
                    return ST_sb[:, j, 0:SC]
                return ST_tl[0:104, 0:SC]

            # ---- GIN layer 1 --------------------------------------------
            # pooled1T[d, i] = sum_j x[j, d] * adjT[j, i]  (+ (1+eps0)*xT)
            p1T = hp.tile([D, N], F32R, tag="p1T")
            for c0, cn in CH:
                q = pmm.tile([D, 512], F32, tag="mm")
                for j in range(NT):
                    nc.tensor.matmul(
                        q[0:D, 0:cn],
                        lhsT=x_blk(j),
                        rhs=adjT_blk(j, c0, cn),
                        start=(j == 0),
                        stop=(j == NT - 1),
                    )
                nc.vector.scalar_tensor_tensor(
                    out=p1T[:, c0 : c0 + cn],
                    in0=xT_sb[:, c0 : c0 + cn],
                    scalar=eps0[0:D, :],
                    in1=q[0:D, 0:cn],
                    op0=ALU.mult,
                    op1=ALU.add,
                )

            # relu1T = relu(w1a^T @ p1T + b1a)
            r1T = hp.tile([128, 2, N], F32R, tag="r1T")
            for m in range(2):
                for c0, cn in CH:
                    q = pmm.tile([128, 512], F32, tag="mm")
                    nc.tensor.matmul(
                        q[:, 0:cn],
                        lhsT=w1a[:, m * 128 : (m + 1) * 128],
                        rhs=p1T[:, c0 : c0 + cn],
                        start=True,
                        stop=True,
                    )
                    nc.scalar.activation(
                        out=r1T[:, m, c0 : c0 + cn],
                        in_=q[:, 0:cn],
                        func=AF.Relu,
                        bias=gin_b["b1a"][:, m : m + 1],
                    )

            # h1T = relu(w1b^T @ r1T + b1b); h1 node-major via PE transpose
            h1T = hp.tile([128, 2, N], BF16, tag="h1T")
            for m in range(2):
                for c0, cn in CH:
                    q = pmm.tile([128, 512], F32, tag="mm")
                    for k in range(2):
                        nc.tensor.matmul(
                            q[:, 0:cn],
                            lhsT=gin_w["w1b"][:, k, m * 128 : (m + 1) * 128],
                            rhs=r1T[:, k, c0 : c0 + cn],
                            start=(k == 0),
                            stop=(k == 1),
                        )
                    nc.scalar.activation(
                        out=h1T[:, m, c0 : c0 + cn],
                        in_=q[:, 0:cn],
                        func=AF.Relu,
                        bias=gin_b["b1b"][:, m : m + 1],
                    )
            h1nm = hp.tile([128, NT, H], BF16, tag="h1nm")
            for m in range(2):
                for j in range(NT):
                    tsz = TS[j]
                    tq = ptp.tile([128, 128], BF16, tag="tp")
                    nc.tensor.transpose(
                        tq[0:tsz, 0:128],
                        in_=h1T[:, m, j * 128 : j * 128 + tsz],
                        identity=ident[:, :],
                    )
                    nc.vector.tensor_copy(
                        out=h1nm[0:tsz, j, m * 128 : (m + 1) * 128],
                        in_=tq[0:tsz, 0:128],
                    )

            # ---- GIN layer 2 --------------------------------------------
            p2T = hp.tile([128, 2, N], F32R, tag="p2T")
            for m in range(2):
                for c0, cn in CH:
                    q = pmm.tile([128, 512], F32, tag="mm")
                    for j in range(NT):
                        nc.tensor.matmul(
                            q[:, 0:cn],
                            lhsT=h1nm[0 : TS[j], j, m * 128 : (m + 1) * 128],
                            rhs=adjT_blk(j, c0, cn),
                            start=(j == 0),
                            stop=(j == NT - 1),
                        )
                    nc.vector.scalar_tensor_tensor(
                        out=p2T[:, m, c0 : c0 + cn],
                        in0=h1T[:, m, c0 : c0 + cn],
                        scalar=eps1[:, :],
                        in1=q[:, 0:cn],
                        op0=ALU.mult,
                        op1=ALU.add,
                    )

            r2T = hp.tile([128, 2, N], F32R, tag="r2T")
            for m in range(2):
                for c0, cn in CH:
                    q = pmm.tile([128, 512], F32, tag="mm")
                    for k in range(2):
                        nc.tensor.matmul(
                            q[:, 0:cn],
                            lhsT=gin_w["w2a"][:, k, m * 128 : (m + 1) * 128],
                            rhs=p2T[:, k, c0 : c0 + cn],
                            start=(k == 0),
                            stop=(k == 1),
                        )
                    nc.scalar.activation(
                        out=r2T[:, m, c0 : c0 + cn],
                        in_=q[:, 0:cn],
                        func=AF.Relu,
                        bias=gin_b["b2a"][:, m : m + 1],
                    )

            h2T = hp.tile([128, 2, N], BF16, tag="h2T")
            for m in range(2):
                for c0, cn in CH:
                    q = pmm.tile([128, 512], F32, tag="mm")
                    for k in range(2):
                        nc.tensor.matmul(
                            q[:, 0:cn],
                            lhsT=gin_w["w2b"][:, k, m * 128 : (m + 1) * 128],
                            rhs=r2T[:, k, c0 : c0 + cn],
                            start=(k == 0),
                            stop=(k == 1),
                        )
                    nc.scalar.activation(
                        out=h2T[:, m, c0 : c0 + cn],
                        in_=q[:, 0:cn],
                        func=AF.Relu,
                        bias=gin_b["b2b"][:, m : m + 1],
                    )
            h2nm = hp.tile([128, NT, H], BF16, tag="h2nm")
            for m in range(2):
                for j in range(NT):
                    tsz = TS[j]
                    tq = ptp.tile([128, 128], BF16, tag="tp")
                    nc.tensor.transpose(
                        tq[0:tsz, 0:128],
                        in_=h2T[:, m, j * 128 : j * 128 + tsz],
                        identity=ident[:, :],
                    )
                    nc.vector.tensor_copy(
                        out=h2nm[0:tsz, j, m * 128 : (m + 1) * 128],
                        in_=tq[0:tsz, 0:128],
                    )

            # ---- candidate gather + graph pool (one matmul) --------------
            # cfT[d, c] = sum_n h2[n, d] * ST[n, c]; col 100 = h_pooled
            cfT = hp.tile([128, 2, SC], BF16, tag="cfT")
            hp32 = hp.tile([128, 2], F32, tag="hp32")
            for m in range(2):
                q = ptp.tile([128, SC], F32, tag="tp")
                for j in range(NT):
                    nc.tensor.matmul(
                        q[:, 0:SC],
                        lhsT=h2nm[0 : TS[j], j, m * 128 : (m + 1) * 128],
                        rhs=ST_blk(j),
                        start=(j == 0),
                        stop=(j == NT - 1),
                    )
                nc.scalar.copy(out=cfT[:, m, :], in_=q[:, 0:SC])
                nc.scalar.copy(out=hp32[:, m : m + 1], in_=q[:, 100:101])

            # ---- actor bias u = wa1p^T @ h_pooled + ba1 ------------------
            qu = psm.tile([HA, 1], F32, tag="qu")
            for k in range(2):
                nc.tensor.matmul(
                    qu[:, :],
                    lhsT=wa1p[:, k, :],
                    rhs=hp32[:, k : k + 1],
                    start=(k == 0),
                    stop=(k == 1),
                )
            ua = hp.tile([HA, 1], F32, tag="ua")
            nc.vector.tensor_add(out=ua[:, :], in0=qu[:, :], in1=ba1[:, :])

            # ---- actor layer 1: cand(rep10) + mach + bias ----------------
            a1T = hp.tile([HA, N], BF16, tag="a1T")
            for c0, cn in CHA:
                q = pmm.tile([HA, 512], F32, tag="mm")
                for k in range(2):
                    src = cfT[:, k, c0 // NM : (c0 + cn) // NM]
                    nc.tensor.matmul(
                        q[:, 0:cn],
                        lhsT=wa1c[:, k, :],
                        rhs=_rep10_ap(src),
                        start=(k == 0),
                        stop=False,
                    )
                nc.tensor.matmul(
                    q[:, 0:cn],
                    lhsT=wa1m[:, :],
                    rhs=machT[:, c0 : c0 + cn],
                    start=False,
                    stop=True,
                )
                nc.scalar.activation(
                    out=a1T[:, c0 : c0 + cn],
                    in_=q[:, 0:cn],
                    func=AF.Tanh,
                    bias=ua[:, :],
                )

            # ---- actor layer 2 ------------------------------------------
            a2T = hp.tile([HA, N], BF16, tag="a2T")
            for c0, cn in CHA:
                q = pmm.tile([HA, 512], F32, tag="mm")
                nc.tensor.matmul(
                    q[:, 0:cn],
                    lhsT=wa2[:, :],
                    rhs=a1T[:, c0 : c0 + cn],
                    start=True,
                    stop=True,
                )
                nc.scalar.activation(
                    out=a2T[:, c0 : c0 + cn],
                    in_=q[:, 0:cn],
                    func=AF.Tanh,
                    bias=ba2[:, :],
                )

            # ---- scores + mask (+ba3 folded into maskneg) ----------------
            sT = hp.tile([1, N], F32, tag="sT")
            for c0, cn in CHA:
                q = pmm.tile([1, 512], F32, tag="mm")
                nc.tensor.matmul(
                    q[0:1, 0:cn],
                    lhsT=wa3[:, :],
                    rhs=a2T[:, c0 : c0 + cn],
                    start=True,
                    stop=True,
                )
                nc.vector.tensor_add(
                    out=sT[:, c0 : c0 + cn],
                    in0=q[0:1, 0:cn],
                    in1=mneg[:, c0 : c0 + cn],
                )

            # ---- masked softmax over the 1000 candidates -----------------
            nmx = hp.tile([1, 1], F32, tag="nmx")
            nc.vector.reduce_max(out=nmx[:, :], in_=sT[:, :], axis=mybir.AxisListType.X, negate=True)
            esb = hp.tile([1, N], F32, tag="esb")
            ssum = hp.tile([1, 1], F32, tag="ssum")
            nc.scalar.activation(
                out=esb[:, :],
                in_=sT[:, :],
                func=AF.Exp,
                bias=nmx[:, :],
                accum_out=ssum[:, :],
            )
            rsum = hp.tile([1, 1], F32, tag="rsum")
            nc.vector.reciprocal(out=rsum[:, :], in_=ssum[:, :])
            pi = hp.tile([1, N], F32, tag="pi")
            nc.vector.tensor_scalar_mul(pi[:, :], in0=esb[:, :], scalar1=rsum[:, :])
            nc.sync.dma_start(out=out_e[b : b + 1, 0:1000], in_=pi[:, :])

            # ---- critic head --------------------------------------------
            qc1 = psm.tile([HA, 1], F32, tag="qu")
            for k in range(2):
                nc.tensor.matmul(
                    qc1[:, :],
                    lhsT=wc1[:, k, :],
                    rhs=hp32[:, k : k + 1],
                    start=(k == 0),
                    stop=(k == 1),
                )
            c1 = hp.tile([HA, 1], F32, tag="c1")
            nc.scalar.activation(out=c1[:, :], in_=qc1[:, :], func=AF.Tanh, bias=bc1[:, :])
            qc2 = psm.tile([HA, 1], F32, tag="qu")
            nc.tensor.matmul(qc2[:, :], lhsT=wc2[:, :], rhs=c1[:, :], start=True, stop=True)
            c2 = hp.tile([HA, 1], F32, tag="c2")
            nc.scalar.activation(out=c2[:, :], in_=qc2[:, :], func=AF.Tanh, bias=bc2[:, :])
            qv = psm.tile([1, 1], F32, tag="qu")
            nc.tensor.matmul(qv[:, :], lhsT=wc3[:, :], rhs=c2[:, :], start=True, stop=True)
            v = hp.tile([1, 1], F32, tag="v")
            nc.scalar.activation(out=v[:, :], in_=qv[:, :], func=AF.Identity, bias=bc3[:, :])
            nc.sync.dma_start(out=out_e[b : b + 1, 1000:1001], in_=v[:, :])

    _split_sync_waits(nc)
    return nc


_NC_CACHE = {}


def _get_nc():
    if "nc" not in _NC_CACHE:
        _NC_CACHE["nc"] = _build_nc()
    return _NC_CACHE["nc"]


def _leaf(a):
    return np.asarray(a)


def _prep_inputs(inputs):
    x = _leaf(inputs["x"]).astype(np.float32)
    adj = _leaf(inputs["adj_matrix"]).astype(np.float32)
    gpool = _leaf(inputs["graph_pool"]).astype(np.float32)
    cand = _leaf(inputs["candidate"])
    mask = _leaf(inputs["mask"])
    mach = _leaf(inputs["machine_feat"]).astype(np.float32)
    gin_params = [[(_leaf(w), _leaf(bb)) for (w, bb) in layer] for layer in inputs["gin_params"]]
    eps = _leaf(inputs["eps"]).astype(np.float32)
    actor = [(_leaf(w), _leaf(bb)) for (w, bb) in inputs["actor_params"]]
    critic = [(_leaf(w), _leaf(bb)) for (w, bb) in inputs["critic_params"]]

    # torch.unique semantics (jnp.unique size=NJ fill=0): sorted unique,
    # truncated/padded to NJ
    cand0 = cand[:, :, 0].astype(np.int64)
    cand_ops = np.zeros((B, NJ), np.int64)
    for bb in range(B):
        u = np.unique(cand0[bb])
        if len(u) >= NJ:
            cand_ops[bb] = u[:NJ]
        else:
            cand_ops[bb, : len(u)] = u
    # one-hot gather matrix, graph_pool packed as column 100
    ST = np.zeros((B, N, SC), np.float32)
    bidx = np.repeat(np.arange(B), NJ)
    ST[bidx, cand_ops.reshape(-1), np.tile(np.arange(NJ), B)] = 1.0
    ST[:, :, 100] = gpool

    ba3 = float(np.asarray(actor[2][1]).reshape(-1)[0])
    maskneg = np.where(mask, np.float32(NEG), np.float32(0.0)).astype(np.float32) + ba3

    shared = {
        "eps": eps.reshape(2, 1),
        "w1a": gin_params[0][0][0].astype(np.float32),
        "b1a": np.ascontiguousarray(
            gin_params[0][0][1].astype(np.float32).reshape(2, 128).T
        ),
        "w1b": gin_params[0][1][0].astype(np.float32).reshape(2, 128, H),
        "b1b": np.ascontiguousarray(
            gin_params[0][1][1].astype(np.float32).reshape(2, 128).T
        ),
        "w2a": gin_params[1][0][0].astype(np.float32).reshape(2, 128, H),
        "b2a": np.ascontiguousarray(
            gin_params[1][0][1].astype(np.float32).reshape(2, 128).T
        ),
        "w2b": gin_params[1][1][0].astype(np.float32).reshape(2, 128, H),
        "b2b": np.ascontiguousarray(
            gin_params[1][1][1].astype(np.float32).reshape(2, 128).T
        ),
        "wa1c": np.ascontiguousarray(actor[0][0][0:256]).astype(_nbf16).reshape(2, 128, HA),
        "wa1m": np.ascontiguousarray(actor[0][0][256:260]).astype(_nbf16),
        "wa1p": np.ascontiguousarray(actor[0][0][260:516]).astype(np.float32).reshape(2, 128, HA),
        "ba1": actor[0][1].astype(np.float32).reshape(HA, 1),
        "wa2": actor[1][0].astype(_nbf16),
        "ba2": actor[1][1].astype(np.float32).reshape(HA, 1),
        "wa3": actor[2][0].astype(_nbf16),
        "wc1": critic[0][0].astype(np.float32).reshape(2, 128, HA),
        "bc1": critic[0][1].astype(np.float32).reshape(HA, 1),
        "wc2": critic[1][0].astype(np.float32),
        "bc2": critic[1][1].astype(np.float32).reshape(HA, 1),
        "wc3": critic[2][0].astype(np.float32),
        "bc3": critic[2][1].astype(np.float32).reshape(1, 1),
    }

    adj_bf = adj.astype(_nbf16)
    x_bf = x.astype(_nbf16)
    mach_bf = mach.astype(_nbf16)
    ST_bf = ST.astype(_nbf16)

    in_maps = []
    for i in range(N_CORES):
        sl = slice(i * BPC, (i + 1) * BPC)
        m = dict(shared)
        m["adjT"] = np.ascontiguousarray(adj_bf[sl].transpose(0, 2, 1))
        m["x"] = np.ascontiguousarray(x_bf[sl])
        m["xT"] = np.ascontiguousarray(x_bf[sl].transpose(0, 2, 1))
        m["ST"] = np.ascontiguousarray(ST_bf[sl])
        m["machT"] = np.ascontiguousarray(mach_bf[sl].transpose(0, 2, 1))
        m["maskneg"] = np.ascontiguousarray(maskneg[sl].reshape(BPC, 1, N))
        in_maps.append(m)
    return in_maps


def _run(inputs, trace=False):
    in_maps = _prep_inputs(inputs)
    nc = _get_nc()
    res = run_bass_kernel_spmd(
        nc, in_maps, core_ids=list(range(N_CORES)), trace=trace
    )
    outs = np.concatenate([np.asarray(res.results[i]["out"]) for i in range(N_CORES)], axis=0)
    pi = outs[:, 0:1000].reshape(B, N, 1).astype(np.float32)
    v = outs[:, 1000:1001].astype(np.float32)
    return pi, v, res.exec_time_ns


def kernel(**inputs):
    pi, v, _ = _run(inputs, trace=False)
    return pi, v


# revision 28
# speedup vs baseline: 1.1898x; 1.0138x over previous
"""Trainium2 Bass kernel for the GIN ActorCritic forward pass.

Shards batch-parallel over 8 NeuronCores (4 graphs each). Host-side
preprocessing: transpose+bf16-cast adjacency, build one-hot candidate
gather matrix (torch.unique semantics) with graph_pool packed as an
extra column, fold actor bias b3 + mask into an additive score mask.
"""
import sys
import types

sys.path.insert(0, "/opt/trn_rl_repo")

import numpy as np
import ml_dtypes

import concourse.bass as bass
import concourse.mybir as mybir
import concourse.tile as tile
from concourse.vector_clock import ScopedClock
from concourse.masks import make_identity
from concourse.bass_utils import run_bass_kernel_spmd

BF16 = mybir.dt.bfloat16
F32 = mybir.dt.float32
F32R = mybir.dt.float32r
AF = mybir.ActivationFunctionType
ALU = mybir.AluOpType

B, N, D, H, HA = 32, 1000, 8, 256, 64
NJ, NM = 100, 10
N_CORES = 8
BPC = B // N_CORES  # 4 graphs per core
SC = 104  # ST columns: 100 one-hot cand cols + col 100 = graph_pool + pad
NT = 8  # node tiles of 128 (last is 104)
TS = [128] * 7 + [104]
CH = [(0, 512), (512, 488)]  # free-dim chunks for GIN stages
CHA = [(0, 500), (500, 500)]  # actor chunks (aligned to cand groups of 10)
NEG = -1.0e30

_nbf16 = ml_dtypes.bfloat16


# ---------------------------------------------------------------------------
# Tile drain patch: walrus in this image rejects >2 sync waits on a CTRL
# drain; split the final global-clock drain into one-wait-per-drain chain.
def _patched_drain_and_barrier(self, tick_clock, wait_clock):
    nc = self.nc
    drain_inst = nc.sync.drain()
    wait_clock.add_sem_waits(
        drain_inst.ins, ScopedClock({None: tick_clock.global_clock})
    )
    waits = list(drain_inst.ins.sync_info.on_wait or [])
    if len(waits) > 1:
        drain_inst.ins.sync_info.on_wait = waits[:1]
        for w in waits[1:]:
            d = nc.sync.drain()
            d.ins.sync_info = mybir.SyncInfo(on_wait=[w], on_update=[])
    nc.all_engine_barrier()
    popped = nc._tile_sem_poison_stack.pop()
    assert popped is self._sem_poison
    nc.clear_and_free_semaphores(list(self.sems.allocated().values()))
    nc.all_engine_barrier()


tile.TileContext._drain_and_barrier = _patched_drain_and_barrier

MAX_WAITS = 1


def _split_sync_waits(nc, max_waits=MAX_WAITS):
    """walrus in this image encodes at most `max_waits` sem-waits per
    instruction; hoist the excess into same-engine NoOps placed just
    before the instruction."""
    n_split = 0
    for f in nc.m.functions:
        for bb in f.blocks:
            insts = list(bb.instructions)
            out = []
            changed = False
            for inst in insts:
                si = inst.sync_info
                waits = list(si.on_wait) if (si is not None and si.on_wait) else []
                if len(waits) > max_waits:
                    changed = True
                    extra = waits[: len(waits) - max_waits]
                    for i in range(0, len(extra), max_waits):
                        chunk = extra[i : i + max_waits]
                        nop = mybir.InstNoOp(
                            name=f"I-wsplit-{n_split}",
                            engine=inst.engine,
                            ins=[],
                            outs=[],
                            sync_info=mybir.SyncInfo(on_wait=chunk, on_update=[]),
                        )
                        n_split += 1
                        out.append(nop)
                    si.on_wait = waits[len(waits) - max_waits :]
                out.append(inst)
            if changed:
                bb.instructions = out
    return n_split


def _install_ntff_shim():
    """Provide the missing antenv.axon_hooks so trace=True works (test.py)."""
    if "antenv.axon_hooks" in sys.modules:
        return
    mod = types.ModuleType("antenv.axon_hooks")
    mod._hook = None
    mod.set_axon_ntff_profile_hook = lambda h: setattr(mod, "_hook", h)
    mod.get_axon_ntff_profile_hook = lambda: mod._hook
    sys.modules["antenv.axon_hooks"] = mod
    import antenv

    antenv.axon_hooks = mod
    try:
        from trn_agent_boot.trn_boot import _ntff_profile_via_ctypes

        mod.set_axon_ntff_profile_hook(
            _ntff_profile_via_ctypes("/opt/axon/libaxon_pjrt.so")
        )
    except Exception:
        pass


def _bcast_ap(ap, count=128):
    """Partition-broadcast a [1,1]-style dram element to `count` partitions."""
    return bass.AP(tensor=ap.tensor, offset=ap.offset, ap=[[0, count]] + list(ap.ap))


def _rep10_ap(ap):
    """Append an inner stride-0 dim of 10 (repeat_interleave along free)."""
    return bass.AP(
        tensor=ap.tensor, offset=ap.offset, ap=list(ap.ap) + [[0, NM]]
    )


def _build_nc():
    nc = bass.Bass()

    # --- per-core sharded inputs -----------------------------------------
    adjT_e = nc.declare_dram_parameter("adjT", [BPC, N, N], BF16, isOutput=False)
    x_e = nc.declare_dram_parameter("x", [BPC, N, D], BF16, isOutput=False)
    xT_e = nc.declare_dram_parameter("xT", [BPC, D, N], BF16, isOutput=False)
    ST_e = nc.declare_dram_parameter("ST", [BPC, N, SC], BF16, isOutput=False)
    machT_e = nc.declare_dram_parameter("machT", [BPC, 4, N], BF16, isOutput=False)
    mneg_e = nc.declare_dram_parameter("maskneg", [BPC, 1, N], F32, isOutput=False)
    # --- replicated weights ----------------------------------------------
    eps_e = nc.declare_dram_parameter("eps", [2, 1], F32, isOutput=False)
    w1a_e = nc.declare_dram_parameter("w1a", [D, H], F32R, isOutput=False)
    w1b_e = nc.declare_dram_parameter("w1b", [2, 128, H], F32R, isOutput=False)
    w2a_e = nc.declare_dram_parameter("w2a", [2, 128, H], F32R, isOutput=False)
    w2b_e = nc.declare_dram_parameter("w2b", [2, 128, H], F32R, isOutput=False)
    b1a_e = nc.declare_dram_parameter("b1a", [128, 2], F32, isOutput=False)
    b1b_e = nc.declare_dram_parameter("b1b", [128, 2], F32, isOutput=False)
    b2a_e = nc.declare_dram_parameter("b2a", [128, 2], F32, isOutput=False)
    b2b_e = nc.declare_dram_parameter("b2b", [128, 2], F32, isOutput=False)
    wa1c_e = nc.declare_dram_parameter("wa1c", [2, 128, HA], BF16, isOutput=False)
    wa1p_e = nc.declare_dram_parameter("wa1p", [2, 128, HA], F32, isOutput=False)
    wa1m_e = nc.declare_dram_parameter("wa1m", [4, HA], BF16, isOutput=False)
    ba1_e = nc.declare_dram_parameter("ba1", [HA, 1], F32, isOutput=False)
    wa2_e = nc.declare_dram_parameter("wa2", [HA, HA], BF16, isOutput=False)
    ba2_e = nc.declare_dram_parameter("ba2", [HA, 1], F32, isOutput=False)
    wa3_e = nc.declare_dram_parameter("wa3", [HA, 1], BF16, isOutput=False)
    wc1_e = nc.declare_dram_parameter("wc1", [2, 128, HA], F32, isOutput=False)
    bc1_e = nc.declare_dram_parameter("bc1", [HA, 1], F32, isOutput=False)
    wc2_e = nc.declare_dram_parameter("wc2", [HA, HA], F32, isOutput=False)
    bc2_e = nc.declare_dram_parameter("bc2", [HA, 1], F32, isOutput=False)
    wc3_e = nc.declare_dram_parameter("wc3", [HA, 1], F32, isOutput=False)
    bc3_e = nc.declare_dram_parameter("bc3", [1, 1], F32, isOutput=False)
    out_e = nc.declare_dram_parameter("out", [BPC, 1001], F32, isOutput=True)

    from contextlib import ExitStack

    with tile.TileContext(nc) as tc, ExitStack() as ctx:
        wp = ctx.enter_context(tc.tile_pool(name="wp", bufs=1))
        ap_ = ctx.enter_context(tc.tile_pool(name="adj", bufs=2))
        sp = ctx.enter_context(tc.tile_pool(name="small", bufs=2))
        hp = ctx.enter_context(tc.tile_pool(name="acts", bufs=2))
        pmm = ctx.enter_context(tc.tile_pool(name="pmm", bufs=4, space="PSUM"))
        ptp = ctx.enter_context(tc.tile_pool(name="ptp", bufs=2, space="PSUM"))
        psm = ctx.enter_context(tc.tile_pool(name="psm", bufs=1, space="PSUM"))

        # ---- constants & weights (loaded once) --------------------------
        ident = wp.tile([128, 128], BF16)
        make_identity(nc, ident[:, :])

        eps0 = wp.tile([128, 1], F32, tag="eps0")
        eps1 = wp.tile([128, 1], F32, tag="eps1")
        e_ap = eps_e[:, :]
        nc.sync.dma_start(
            out=eps0[:, :],
            in_=bass.AP(tensor=e_ap.tensor, offset=e_ap.offset, ap=[[0, 128], [1, 1]]),
        )
        nc.sync.dma_start(
            out=eps1[:, :],
            in_=bass.AP(
                tensor=e_ap.tensor, offset=e_ap.offset + 1, ap=[[0, 128], [1, 1]]
            ),
        )
        # 1 + eps
        nc.scalar.add(out=eps0[:, :], in_=eps0[:, :], add=1.0)
        nc.scalar.add(out=eps1[:, :], in_=eps1[:, :], add=1.0)

        w1a = wp.tile([D, H], F32R, tag="w1a")
        nc.sync.dma_start(out=w1a[:, :], in_=w1a_e[:, :])
        gin_w = {}
        for nm, ext in (("w1b", w1b_e), ("w2a", w2a_e), ("w2b", w2b_e)):
            t = wp.tile([128, 2, H], F32R, tag=nm)
            for k in range(2):
                nc.sync.dma_start(out=t[:, k, :], in_=ext[k])
            gin_w[nm] = t
        gin_b = {}
        for nm, ext in (
            ("b1a", b1a_e),
            ("b1b", b1b_e),
            ("b2a", b2a_e),
            ("b2b", b2b_e),
        ):
            t = wp.tile([128, 2], F32, tag=nm)
            nc.sync.dma_start(out=t[:, :], in_=ext[:, :])
            gin_b[nm] = t
        wa1c = wp.tile([128, 2, HA], BF16, tag="wa1c")
        wa1p = wp.tile([128, 2, HA], F32, tag="wa1p")
        wc1 = wp.tile([128, 2, HA], F32, tag="wc1")
        for t, ext in ((wa1c, wa1c_e), (wa1p, wa1p_e), (wc1, wc1_e)):
            for k in range(2):
                nc.sync.dma_start(out=t[:, k, :], in_=ext[k])
        wa1m = wp.tile([4, HA], BF16, tag="wa1m")
        nc.sync.dma_start(out=wa1m[:, :], in_=wa1m_e[:, :])
        wa2 = wp.tile([HA, HA], BF16, tag="wa2")
        nc.sync.dma_start(out=wa2[:, :], in_=wa2_e[:, :])
        wa3 = wp.tile([HA, 1], BF16, tag="wa3")
        nc.sync.dma_start(out=wa3[:, :], in_=wa3_e[:, :])
        wc2 = wp.tile([HA, HA], F32, tag="wc2")
        nc.sync.dma_start(out=wc2[:, :], in_=wc2_e[:, :])
        wc3 = wp.tile([HA, 1], F32, tag="wc3")
        nc.sync.dma_start(out=wc3[:, :], in_=wc3_e[:, :])
        ba1 = wp.tile([HA, 1], F32, tag="ba1")
        nc.sync.dma_start(out=ba1[:, :], in_=ba1_e[:, :])
        ba2 = wp.tile([HA, 1], F32, tag="ba2")
        nc.sync.dma_start(out=ba2[:, :], in_=ba2_e[:, :])
        bc1 = wp.tile([HA, 1], F32, tag="bc1")
        nc.sync.dma_start(out=bc1[:, :], in_=bc1_e[:, :])
        bc2 = wp.tile([HA, 1], F32, tag="bc2")
        nc.sync.dma_start(out=bc2[:, :], in_=bc2_e[:, :])
        bc3 = wp.tile([1, 1], F32, tag="bc3")
        nc.sync.dma_start(out=bc3[:, :], in_=bc3_e[:, :])

        for b in range(BPC):
            # ---- load per-batch inputs ----------------------------------
            adjT = ap_.tile([128, 7, N], BF16, tag="adjT")
            adjTt = ap_.tile([128, N], BF16, tag="adjTt")
            nc.sync.dma_start(
                out=adjT[:, :, :],
                in_=adjT_e[b, 0:896, :].rearrange("(j p) i -> p j i", p=128),
            )
            nc.sync.dma_start(out=adjTt[0:104, :], in_=adjT_e[b, 896:1000, :])

            x_sb = sp.tile([128, 7, D], BF16, tag="x")
            x_tl = sp.tile([128, D], BF16, tag="xt")
            nc.sync.dma_start(
                out=x_sb[:, :, :],
                in_=x_e[b, 0:896, :].rearrange("(j p) d -> p j d", p=128),
            )
            nc.sync.dma_start(out=x_tl[0:104, :], in_=x_e[b, 896:1000, :])

            ST_sb = sp.tile([128, 7, SC], BF16, tag="ST")
            ST_tl = sp.tile([128, SC], BF16, tag="STt")
            nc.sync.dma_start(
                out=ST_sb[:, :, :],
                in_=ST_e[b, 0:896, :].rearrange("(j p) c -> p j c", p=128),
            )
            nc.sync.dma_start(out=ST_tl[0:104, :], in_=ST_e[b, 896:1000, :])

            xT_sb = sp.tile([D, N], BF16, tag="xT")
            nc.sync.dma_start(out=xT_sb[:, :], in_=xT_e[b])
            machT = sp.tile([4, N], BF16, tag="machT")
            nc.sync.dma_start(out=machT[:, :], in_=machT_e[b])
            mneg = sp.tile([1, N], F32, tag="mneg")
            nc.sync.dma_start(out=mneg[:, :], in_=mneg_e[b])

            def adjT_blk(j, c0, cn):
                if j < 7:
                    return adjT[:, j, c0 : c0 + cn]
                return adjTt[0:104, c0 : c0 + cn]

            def x_blk(j):
                if j < 7:
                    return x_sb[:, j, :]
                return x_tl[0:104, :]

            def ST_blk(j):
                if j < 7:
                    return ST_sb[:, j, 0:SC]
                return ST_tl[0:104, 0:SC]

            # ---- GIN layer 1 --------------------------------------------
            # pooled1T[d, i] = sum_j x[j, d] * adjT[j, i]  (+ (1+eps0)*xT)
            p1T = hp.tile([D, N], F32R, tag="p1T")
            for c0, cn in CH:
                q = pmm.tile([D, 512], F32, tag="mm")
                for j in range(NT):
                    nc.tensor.matmul(
                        q[0:D, 0:cn],
                        lhsT=x_blk(j),
                        rhs=adjT_blk(j, c0, cn),
                        start=(j == 0),
                        stop=(j == NT - 1),
                    )
                nc.vector.scalar_tensor_tensor(
                    out=p1T[:, c0 : c0 + cn],
                    in0=xT_sb[:, c0 : c0 + cn],
                    scalar=eps0[0:D, :],
                    in1=q[0:D, 0:cn],
                    op0=ALU.mult,
                    op1=ALU.add,
                )

            # relu1T = relu(w1a^T @ p1T + b1a)
            r1T = hp.tile([128, 2, N], F32R, tag="r1T")
            for m in range(2):
                for c0, cn in CH:
                    q = pmm.tile([128, 512], F32, tag="mm")
                    nc.tensor.matmul(
                        q[:, 0:cn],
                        lhsT=w1a[:, m * 128 : (m + 1) * 128],
                        rhs=p1T[:, c0 : c0 + cn],
                        start=True,
                        stop=True,
                    )
                    nc.scalar.activation(
                        out=r1T[:, m, c0 : c0 + cn],
                        in_=q[:, 0:cn],
                        func=AF.Relu,
                        bias=gin_b["b1a"][:, m : m + 1],
                    )

            # h1T = relu(w1b^T @ r1T + b1b); h1 node-major via PE transpose
            h1T = hp.tile([128, 2, N], BF16, tag="h1T")
            for m in range(2):
                for c0, cn in CH:
                    q = pmm.tile([128, 512], F32, tag="mm")
                    for k in range(2):
                        nc.tensor.matmul(
                            q[:, 0:cn],
                            lhsT=gin_w["w1b"][:, k, m * 128 : (m + 1) * 128],
                            rhs=r1T[:, k, c0 : c0 + cn],
                            start=(k == 0),
                            stop=(k == 1),
                        )
                    nc.scalar.activation(
                        out=h1T[:, m, c0 : c0 + cn],
                        in_=q[:, 0:cn],
                        func=AF.Relu,
                        bias=gin_b["b1b"][:, m : m + 1],
                    )
            h1nm = hp.tile([128, NT, H], BF16, tag="h1nm")
            for m in range(2):
                for j in range(NT):
                    tsz = TS[j]
                    tq = ptp.tile([128, 128], BF16, tag="tp")
                    nc.tensor.transpose(
                        tq[0:tsz, 0:128],
                        in_=h1T[:, m, j * 128 : j * 128 + tsz],
                        identity=ident[:, :],
                    )
                    nc.vector.tensor_copy(
                        out=h1nm[0:tsz, j, m * 128 : (m + 1) * 128],
                        in_=tq[0:tsz, 0:128],
                    )

            # ---- GIN layer 2 --------------------------------------------
            p2T = hp.tile([128, 2, N], F32R, tag="p2T")
            for m in range(2):
                for c0, cn in CH:
                    q = pmm.tile([128, 512], F32, tag="mm")
                    for j in range(NT):
                        nc.tensor.matmul(
                            q[:, 0:cn],
                            lhsT=h1nm[0 : TS[j], j, m * 128 : (m + 1) * 128],
                            rhs=adjT_blk(j, c0, cn),
                            start=(j == 0),
                            stop=(j == NT - 1),
                        )
                    nc.vector.scalar_tensor_tensor(
                        out=p2T[:, m, c0 : c0 + cn],
                        in0=h1T[:, m, c0 : c0 + cn],
                        scalar=eps1[:, :],
                        in1=q[:, 0:cn],
                        op0=ALU.mult,
                        op1=ALU.add,
                    )

            r2T = hp.tile([128, 2, N], F32R, tag="r2T")
            for m in range(2):
                for c0, cn in CH:
                    q = pmm.tile([128, 512], F32, tag="mm")
                    for k in range(2):
                        nc.tensor.matmul(
                            q[:, 0:cn],
                            lhsT=gin_w["w2a"][:, k, m * 128 : (m + 1) * 128],
                            rhs=p2T[:, k, c0 : c0 + cn],
                            start=(k == 0),
                            stop=(k == 1),
                        )
                    nc.scalar.activation(
                        out=r2T[:, m, c0 : c0 + cn],
                        in_=q[:, 0:cn],
                        func=AF.Relu,
                        bias=gin_b["b2a"][:, m : m + 1],
                    )

            h2T = hp.tile([128, 2, N], BF16, tag="h2T")
            for m in range(2):
                for c0, cn in CH:
                    q = pmm.tile([128, 512], F32, tag="mm")
                    for k in range(2):
                        nc.tensor.matmul(
                            q[:, 0:cn],
                            lhsT=gin_w["w2b"][:, k, m * 128 : (m + 1) * 128],
                            rhs=r2T[:, k, c0 : c0 + cn],
                            start=(k == 0),
                            stop=(k == 1),
                        )
                    nc.scalar.activation(
                        out=h2T[:, m, c0 : c0 + cn],
                        in_=q[:, 0:cn],
                        func=AF.Relu,
                        bias=gin_b["b2b"][:, m : m + 1],
                    )
            h2nm = hp.tile([128, NT, H], BF16, tag="h2nm")
            for m in range(2):
                for j in range(NT):
                    tsz = TS[j]
                    tq = ptp.tile([128, 128], BF16, tag="tp")
                    nc.tensor.transpose(
                        tq[0:tsz, 0:128],
                        in_=h2T[:, m, j * 128 : j * 128 + tsz],
                        identity=ident[:, :],
                    )
                    nc.vector.tensor_copy(
                        out=h2nm[0:tsz, j, m * 128 : (m + 1) * 128],
                        in_=tq[0:tsz, 0:128],
                    )

            # ---- candidate gather + graph pool (one matmul) --------------
            # cfT[d, c] = sum_n h2[n, d] * ST[n, c]; col 100 = h_pooled
            cfT = hp.tile([128, 2, SC], BF16, tag="cfT")
            hp32 = hp.tile([128, 2], F32, tag="hp32")
            for m in range(2):
                q = ptp.tile([128, SC], F32, tag="tp")
                for j in range(NT):
                    nc.tensor.matmul(
                        q[:, 0:SC],
                        lhsT=h2nm[0 : TS[j], j, m * 128 : (m + 1) * 128],
                        rhs=ST_blk(j),
                        start=(j == 0),
                        stop=(j == NT - 1),
                    )
                nc.scalar.copy(out=cfT[:, m, :], in_=q[:, 0:SC])
                nc.scalar.copy(out=hp32[:, m : m + 1], in_=q[:, 100:101])

            # ---- actor bias u = wa1p^T @ h_pooled + ba1 ------------------
            qu = psm.tile([HA, 1], F32, tag="qu")
            for k in range(2):
                nc.tensor.matmul(
                    qu[:, :],
                    lhsT=wa1p[:, k, :],
                    rhs=hp32[:, k : k + 1],
                    start=(k == 0),
                    stop=(k == 1),
                )
            ua = hp.tile([HA, 1], F32, tag="ua")
            nc.vector.tensor_add(out=ua[:, :], in0=qu[:, :], in1=ba1[:, :])

            # ---- actor layer 1: cand(rep10) + mach + bias ----------------
            a1T = hp.tile([HA, N], BF16, tag="a1T")
            for c0, cn in CHA:
                q = pmm.tile([HA, 512], F32, tag="mm")
                for k in range(2):
                    src = cfT[:, k, c0 // NM : (c0 + cn) // NM]
                    nc.tensor.matmul(
                        q[:, 0:cn],
                        lhsT=wa1c[:, k, :],
                        rhs=_rep10_ap(src),
                        start=(k == 0),
                        stop=False,
                    )
                nc.tensor.matmul(
                    q[:, 0:cn],
                    lhsT=wa1m[:, :],
                    rhs=machT[:, c0 : c0 + cn],
                    start=False,
                    stop=True,
                )
                nc.scalar.activation(
                    out=a1T[:, c0 : c0 + cn],
                    in_=q[:, 0:cn],
                    func=AF.Tanh,
                    bias=ua[:, :],
                )

            # ---- actor layer 2 ------------------------------------------
            a2T = hp.tile([HA, N], BF16, tag="a2T")
            for c0, cn in CHA:
                q = pmm.tile([HA, 512], F32, tag="mm")
                nc.tensor.matmul(
                    q[:, 0:cn],
                    lhsT=wa2[:, :],
                    rhs=a1T[:, c0 : c0 + cn],
                    start=True,
                    stop=True,
                )
                nc.scalar.activation(
                    out=a2T[:, c0 : c0 + cn],
                    in_=q[:, 0:cn],
                    func=AF.Tanh,
                    bias=ba2[:, :],
                )

            # ---- scores + mask (+ba3 folded into maskneg) ----------------
            sT = hp.tile([1, N], F32, tag="sT")
            for c0, cn in CHA:
                q = pmm.tile([1, 512], F32, tag="mm")
                nc.tensor.matmul(
                    q[0:1, 0:cn],
                    lhsT=wa3[:, :],
                    rhs=a2T[:, c0 : c0 + cn],
                    start=True,
                    stop=True,
                )
                nc.vector.tensor_add(
                    out=sT[:, c0 : c0 + cn],
                    in0=q[0:1, 0:cn],
                    in1=mneg[:, c0 : c0 + cn],
                )

            # ---- masked softmax over the 1000 candidates -----------------
            nmx = hp.tile([1, 1], F32, tag="nmx")
            nc.vector.reduce_max(out=nmx[:, :], in_=sT[:, :], axis=mybir.AxisListType.X, negate=True)
            esb = hp.tile([1, N], F32, tag="esb")
            ssum = hp.tile([1, 1], F32, tag="ssum")
            nc.scalar.activation(
                out=esb[:, :],
                in_=sT[:, :],
                func=AF.Exp,
                bias=nmx[:, :],
                accum_out=ssum[:, :],
            )
            rsum = hp.tile([1, 1], F32, tag="rsum")
            nc.vector.reciprocal(out=rsum[:, :], in_=ssum[:, :])
            pi = hp.tile([1, N], F32, tag="pi")
            nc.vector.tensor_scalar_mul(pi[:, :], in0=esb[:, :], scalar1=rsum[:, :])
            nc.sync.dma_start(out=out_e[b : b + 1, 0:1000], in_=pi[:, :])

            # ---- critic head --------------------------------------------
            qc1 = psm.tile([HA, 1], F32, tag="qu")
            for k in range(2):
                nc.tensor.matmul(
                    qc1[:, :],
                    lhsT=wc1[:, k, :],
                    rhs=hp32[:, k : k + 1],
                    start=(k == 0),
                    stop=(k == 1),
                )
            c1 = hp.tile([HA, 1], F32, tag="c1")
            nc.scalar.activation(out=c1[:, :], in_=qc1[:, :], func=AF.Tanh, bias=bc1[:, :])
            qc2 = psm.tile([HA, 1], F32, tag="qu")
            nc.tensor.matmul(qc2[:, :], lhsT=wc2[:, :], rhs=c1[:, :], start=True, stop=True)
            c2 = hp.tile([HA, 1], F32, tag="c2")
            nc.scalar.activation(out=c2[:, :], in_=qc2[:, :], func=AF.Tanh, bias=bc2[:, :])
            qv = psm.tile([1, 1], F32, tag="qu")
            nc.tensor.matmul(qv[:, :], lhsT=wc3[:, :], rhs=c2[:, :], start=True, stop=True)
            v = hp.tile([1, 1], F32, tag="v")
            nc.scalar.activation(out=v[:, :], in_=qv[:, :], func=AF.Identity, bias=bc3[:, :])
            nc.sync.dma_start(out=out_e[b : b + 1, 1000:1001], in_=v[:, :])

    _split_sync_waits(nc)
    return nc


_NC_CACHE = {}


def _get_nc():
    if "nc" not in _NC_CACHE:
        _NC_CACHE["nc"] = _build_nc()
    return _NC_CACHE["nc"]


def _leaf(a):
    return np.asarray(a)


def _prep_inputs(inputs):
    x = _leaf(inputs["x"]).astype(np.float32)
    adj = _leaf(inputs["adj_matrix"]).astype(np.float32)
    gpool = _leaf(inputs["graph_pool"]).astype(np.float32)
    cand = _leaf(inputs["candidate"])
    mask = _leaf(inputs["mask"])
    mach = _leaf(inputs["machine_feat"]).astype(np.float32)
    gin_params = [[(_leaf(w), _leaf(bb)) for (w, bb) in layer] for layer in inputs["gin_params"]]
    eps = _leaf(inputs["eps"]).astype(np.float32)
    actor = [(_leaf(w), _leaf(bb)) for (w, bb) in inputs["actor_params"]]
    critic = [(_leaf(w), _leaf(bb)) for (w, bb) in inputs["critic_params"]]

    # torch.unique semantics (jnp.unique size=NJ fill=0): sorted unique,
    # truncated/padded to NJ
    cand0 = cand[:, :, 0].astype(np.int64)
    cand_ops = np.zeros((B, NJ), np.int64)
    for bb in range(B):
        u = np.unique(cand0[bb])
        if len(u) >= NJ:
            cand_ops[bb] = u[:NJ]
        else:
            cand_ops[bb, : len(u)] = u
    # one-hot gather matrix, graph_pool packed as column 100
    ST = np.zeros((B, N, SC), np.float32)
    bidx = np.repeat(np.arange(B), NJ)
    ST[bidx, cand_ops.reshape(-1), np.tile(np.arange(NJ), B)] = 1.0
    ST[:, :, 100] = gpool

    ba3 = float(np.asarray(actor[2][1]).reshape(-1)[0])
    maskneg = np.where(mask, np.float32(NEG), np.float32(0.0)).astype(np.float32) + ba3

    shared = {
        "eps": eps.reshape(2, 1),
        "w1a": gin_params[0][0][0].astype(np.float32),
        "b1a": np.ascontiguousarray(
            gin_params[0][0][1].astype(np.float32).reshape(2, 128).T
        ),
        "w1b": gin_params[0][1][0].astype(np.float32).reshape(2, 128, H),
        "b1b": np.ascontiguousarray(
            gin_params[0][1][1].astype(np.float32).reshape(2, 128).T
        ),
        "w2a": gin_params[1][0][0].astype(np.float32).reshape(2, 128, H),
        "b2a": np.ascontiguousarray(
            gin_params[1][0][1].astype(np.float32).reshape(2, 128).T
        ),
        "w2b": gin_params[1][1][0].astype(np.float32).reshape(2, 128, H),
        "b2b": np.ascontiguousarray(
            gin_params[1][1][1].astype(np.float32).reshape(2, 128).T
        ),
        "wa1c": np.ascontiguousarray(actor[0][0][0:256]).astype(_nbf16).reshape(2, 128, HA),
        "wa1m": np.ascontiguousarray(actor[0][0][256:260]).astype(_nbf16),
        "wa1p": np.ascontiguousarray(actor[0][0][260:516]).astype(np.float32).reshape(2, 128, HA),
        "ba1": actor[0][1].astype(np.float32).reshape(HA, 1),
        "wa2": actor[1][0].astype(_nbf16),
        "ba2": actor[1][1].astype(np.float32).reshape(HA, 1),
        "wa3": actor[2][0].astype(_nbf16),
        "wc1": critic[0][0].astype(np.float32).reshape(2, 128, HA),
        "bc1": critic[0][1].astype(np.float32).reshape(HA, 1),
        "wc2": critic[1][0].astype(np.float32),
        "bc2": critic[1][1].astype(np.float32).reshape(HA, 1),
        "wc3": critic[2][0].astype(np.float32),
        "bc3": critic[2][1].astype(np.float32).reshape(1, 1),
    }

    adj_bf = adj.astype(_nbf16)
    x_bf = x.astype(_nbf16)
    mach_bf = mach.astype(_nbf16)
    ST_bf = ST.astype(_nbf16)

    in_maps = []
    for i in range(N_CORES):
        sl = slice(i * BPC, (i + 1) * BPC)
        m = dict(shared)
        m["adjT"] = np.ascontiguousarray(adj_bf[sl].transpose(0, 2, 1))
        m["x"] = np.ascontiguousarray(x_bf[sl])
        m["xT"] = np.ascontiguousarray(x_bf[sl].transpose(0, 2, 1))
        m["ST"] = np.ascontiguousarray(ST_bf[sl])
        m["machT"] = np.ascontiguousarray(mach_bf[sl].transpose(0, 2, 1))
        m["maskneg"] = np.ascontiguousarray(maskneg[sl].reshape(BPC, 1, N))
        in_maps.append(m)
    return in_maps


def _run(inputs, trace=False):
    in_maps = _prep_inputs(inputs)
    nc = _get_nc()
    res = run_bass_kernel_spmd(
        nc, in_maps, core_ids=list(range(N_CORES)), trace=trace
    )
    outs = np.concatenate([np.asarray(res.results[i]["out"]) for i in range(N_CORES)], axis=0)
    pi = outs[:, 0:1000].reshape(B, N, 1).astype(np.float32)
    v = outs[:, 1000:1001].astype(np.float32)
    return pi, v, res.exec_time_ns


def kernel(**inputs):
    pi, v, _ = _run(inputs, trace=False)
    return pi, v


# revision 30
# speedup vs baseline: 1.3549x; 1.1387x over previous
"""Trainium2 Bass kernel for the GIN ActorCritic forward pass.

Shards batch-parallel over 8 NeuronCores (4 graphs each). Host-side
preprocessing: transpose+bf16-cast adjacency, build one-hot candidate
gather matrix (torch.unique semantics) with graph_pool packed as an
extra column, fold actor bias b3 + mask into an additive score mask.
"""
import sys
import types

sys.path.insert(0, "/opt/trn_rl_repo")

import numpy as np
import ml_dtypes

import concourse.bass as bass
import concourse.mybir as mybir
import concourse.tile as tile
from concourse.vector_clock import ScopedClock
from concourse.masks import make_identity
from concourse.bass_utils import run_bass_kernel_spmd

BF16 = mybir.dt.bfloat16
F32 = mybir.dt.float32
F32R = mybir.dt.float32r
AF = mybir.ActivationFunctionType
ALU = mybir.AluOpType

B, N, D, H, HA = 32, 1000, 8, 256, 64
NJ, NM = 100, 10
N_CORES = 8
BPC = B // N_CORES  # 4 graphs per core
SC = 104  # ST columns: 100 one-hot cand cols + col 100 = graph_pool + pad
NT = 8  # node tiles of 128 (last is 104)
TS = [128] * 7 + [104]
CH = [(0, 512), (512, 488)]  # free-dim chunks for GIN stages
CHA = [(0, 500), (500, 500)]  # actor chunks (aligned to cand groups of 10)
NEG = -1.0e30

_nbf16 = ml_dtypes.bfloat16


# ---------------------------------------------------------------------------
# Tile drain patch: walrus in this image rejects >2 sync waits on a CTRL
# drain; split the final global-clock drain into one-wait-per-drain chain.
def _patched_drain_and_barrier(self, tick_clock, wait_clock):
    nc = self.nc
    drain_inst = nc.sync.drain()
    wait_clock.add_sem_waits(
        drain_inst.ins, ScopedClock({None: tick_clock.global_clock})
    )
    waits = list(drain_inst.ins.sync_info.on_wait or [])
    if len(waits) > 1:
        drain_inst.ins.sync_info.on_wait = waits[:1]
        for w in waits[1:]:
            d = nc.sync.drain()
            d.ins.sync_info = mybir.SyncInfo(on_wait=[w], on_update=[])
    nc.all_engine_barrier()
    popped = nc._tile_sem_poison_stack.pop()
    assert popped is self._sem_poison
    nc.clear_and_free_semaphores(list(self.sems.allocated().values()))
    nc.all_engine_barrier()


tile.TileContext._drain_and_barrier = _patched_drain_and_barrier

MAX_WAITS = 1


def _split_sync_waits(nc, max_waits=MAX_WAITS):
    """walrus in this image encodes at most `max_waits` sem-waits per
    instruction; hoist the excess into same-engine NoOps placed just
    before the instruction."""
    n_split = 0
    for f in nc.m.functions:
        for bb in f.blocks:
            insts = list(bb.instructions)
            out = []
            changed = False
            for inst in insts:
                si = inst.sync_info
                waits = list(si.on_wait) if (si is not None and si.on_wait) else []
                if len(waits) > max_waits:
                    changed = True
                    extra = waits[: len(waits) - max_waits]
                    for i in range(0, len(extra), max_waits):
                        chunk = extra[i : i + max_waits]
                        nop = mybir.InstNoOp(
                            name=f"I-wsplit-{n_split}",
                            engine=inst.engine,
                            ins=[],
                            outs=[],
                            sync_info=mybir.SyncInfo(on_wait=chunk, on_update=[]),
                        )
                        n_split += 1
                        out.append(nop)
                    si.on_wait = waits[len(waits) - max_waits :]
                out.append(inst)
            if changed:
                bb.instructions = out
    return n_split


def _install_ntff_shim():
    """Provide the missing antenv.axon_hooks so trace=True works (test.py)."""
    if "antenv.axon_hooks" in sys.modules:
        return
    mod = types.ModuleType("antenv.axon_hooks")
    mod._hook = None
    mod.set_axon_ntff_profile_hook = lambda h: setattr(mod, "_hook", h)
    mod.get_axon_ntff_profile_hook = lambda: mod._hook
    sys.modules["antenv.axon_hooks"] = mod
    import antenv

    antenv.axon_hooks = mod
    try:
        from trn_agent_boot.trn_boot import _ntff_profile_via_ctypes

        mod.set_axon_ntff_profile_hook(
            _ntff_profile_via_ctypes("/opt/axon/libaxon_pjrt.so")
        )
    except Exception:
        pass


def _bcast_ap(ap, count=128):
    """Partition-broadcast a [1,1]-style dram element to `count` partitions."""
    return bass.AP(tensor=ap.tensor, offset=ap.offset, ap=[[0, count]] + list(ap.ap))


def _rep10_ap(ap):
    """Append an inner stride-0 dim of 10 (repeat_interleave along free)."""
    return bass.AP(
        tensor=ap.tensor, offset=ap.offset, ap=list(ap.ap) + [[0, NM]]
    )


def _build_nc():
    nc = bass.Bass()

    # --- per-core sharded inputs -----------------------------------------
    adjT_e = nc.declare_dram_parameter("adjT", [BPC, N, N], BF16, isOutput=False)
    x_e = nc.declare_dram_parameter("x", [BPC, N, D], BF16, isOutput=False)
    xT_e = nc.declare_dram_parameter("xT", [BPC, D, N], BF16, isOutput=False)
    ST_e = nc.declare_dram_parameter("ST", [BPC, N, SC], BF16, isOutput=False)
    machT_e = nc.declare_dram_parameter("machT", [BPC, 4, N], BF16, isOutput=False)
    mneg_e = nc.declare_dram_parameter("maskneg", [BPC, 1, N], F32, isOutput=False)
    # --- replicated weights ----------------------------------------------
    eps_e = nc.declare_dram_parameter("eps", [2, 1], F32, isOutput=False)
    w1a_e = nc.declare_dram_parameter("w1a", [D, H], F32R, isOutput=False)
    w1b_e = nc.declare_dram_parameter("w1b", [2, 128, H], F32R, isOutput=False)
    w2a_e = nc.declare_dram_parameter("w2a", [2, 128, H], F32R, isOutput=False)
    w2b_e = nc.declare_dram_parameter("w2b", [2, 128, H], F32R, isOutput=False)
    b1a_e = nc.declare_dram_parameter("b1a", [128, 2], F32, isOutput=False)
    b1b_e = nc.declare_dram_parameter("b1b", [128, 2], F32, isOutput=False)
    b2a_e = nc.declare_dram_parameter("b2a", [128, 2], F32, isOutput=False)
    b2b_e = nc.declare_dram_parameter("b2b", [128, 2], F32, isOutput=False)
    wa1c_e = nc.declare_dram_parameter("wa1c", [2, 128, HA], BF16, isOutput=False)
    wa1p_e = nc.declare_dram_parameter("wa1p", [2, 128, HA], F32, isOutput=False)
    wa1m_e = nc.declare_dram_parameter("wa1m", [4, HA], BF16, isOutput=False)
    ba1_e = nc.declare_dram_parameter("ba1", [HA, 1], F32, isOutput=False)
    wa2_e = nc.declare_dram_parameter("wa2", [HA, HA], BF16, isOutput=False)
    ba2_e = nc.declare_dram_parameter("ba2", [HA, 1], F32, isOutput=False)
    wa3_e = nc.declare_dram_parameter("wa3", [HA, 1], BF16, isOutput=False)
    wc1_e = nc.declare_dram_parameter("wc1", [2, 128, HA], F32, isOutput=False)
    bc1_e = nc.declare_dram_parameter("bc1", [HA, 1], F32, isOutput=False)
    wc2_e = nc.declare_dram_parameter("wc2", [HA, HA], F32, isOutput=False)
    bc2_e = nc.declare_dram_parameter("bc2", [HA, 1], F32, isOutput=False)
    wc3_e = nc.declare_dram_parameter("wc3", [HA, 1], F32, isOutput=False)
    bc3_e = nc.declare_dram_parameter("bc3", [1, 1], F32, isOutput=False)
    out_e = nc.declare_dram_parameter("out", [BPC, 1001], F32, isOutput=True)

    from contextlib import ExitStack

    with tile.TileContext(nc) as tc, ExitStack() as ctx:
        wp = ctx.enter_context(tc.tile_pool(name="wp", bufs=1))
        ap_ = ctx.enter_context(tc.tile_pool(name="adj", bufs=2))
        sp = ctx.enter_context(tc.tile_pool(name="small", bufs=2))
        hp = ctx.enter_context(tc.tile_pool(name="acts", bufs=2))
        pmm = ctx.enter_context(tc.tile_pool(name="pmm", bufs=4, space="PSUM"))
        ptp = ctx.enter_context(tc.tile_pool(name="ptp", bufs=2, space="PSUM"))
        psm = ctx.enter_context(tc.tile_pool(name="psm", bufs=1, space="PSUM"))

        def load_core(b):
            t = {}
            t["adjT"] = ap_.tile([128, 7, N], BF16, tag="adjT", name=f"adjT{b}")
            t["adjTt"] = ap_.tile([128, N], BF16, tag="adjTt", name=f"adjTt{b}")
            t["x_sb"] = sp.tile([128, 7, D], BF16, tag="x", name=f"x{b}")
            t["x_tl"] = sp.tile([128, D], BF16, tag="xt", name=f"xt{b}")
            t["xT_sb"] = sp.tile([D, N], BF16, tag="xT", name=f"xT{b}")
            nc.sync.dma_start(
                out=t["adjT"][:, :, :],
                in_=adjT_e[b, 0:896, :].rearrange("(j p) i -> p j i", p=128),
            )
            nc.sync.dma_start(out=t["adjTt"][0:104, :], in_=adjT_e[b, 896:1000, :])
            nc.sync.dma_start(
                out=t["x_sb"][:, :, :],
                in_=x_e[b, 0:896, :].rearrange("(j p) d -> p j d", p=128),
            )
            nc.sync.dma_start(out=t["x_tl"][0:104, :], in_=x_e[b, 896:1000, :])
            nc.sync.dma_start(out=t["xT_sb"][:, :], in_=xT_e[b])
            return t

        def load_rest(b, t):
            t["ST_sb"] = sp.tile([128, 7, SC], BF16, tag="ST", name=f"ST{b}")
            t["ST_tl"] = sp.tile([128, SC], BF16, tag="STt", name=f"STt{b}")
            nc.sync.dma_start(
                out=t["ST_sb"][:, :, :],
                in_=ST_e[b, 0:896, :].rearrange("(j p) c -> p j c", p=128),
            )
            nc.sync.dma_start(out=t["ST_tl"][0:104, :], in_=ST_e[b, 896:1000, :])
            t["machT"] = sp.tile([4, N], BF16, tag="machT", name=f"machT{b}")
            nc.sync.dma_start(out=t["machT"][:, :], in_=machT_e[b])
            t["mneg"] = sp.tile([1, N], F32, tag="mneg", name=f"mneg{b}")
            nc.sync.dma_start(out=t["mneg"][:, :], in_=mneg_e[b])
            return t

        preload = load_core(0)

        # ---- constants & weights (loaded once) --------------------------
        ident = wp.tile([128, 128], BF16)
        make_identity(nc, ident[:, :])

        eps0 = wp.tile([128, 1], F32, tag="eps0")
        eps1 = wp.tile([128, 1], F32, tag="eps1")
        e_ap = eps_e[:, :]
        nc.sync.dma_start(
            out=eps0[:, :],
            in_=bass.AP(tensor=e_ap.tensor, offset=e_ap.offset, ap=[[0, 128], [1, 1]]),
        )
        nc.sync.dma_start(
            out=eps1[:, :],
            in_=bass.AP(
                tensor=e_ap.tensor, offset=e_ap.offset + 1, ap=[[0, 128], [1, 1]]
            ),
        )
        # 1 + eps
        nc.scalar.add(out=eps0[:, :], in_=eps0[:, :], add=1.0)
        nc.scalar.add(out=eps1[:, :], in_=eps1[:, :], add=1.0)

        w1a = wp.tile([D, H], F32R, tag="w1a")
        nc.sync.dma_start(out=w1a[:, :], in_=w1a_e[:, :])
        gin_w = {}
        for nm, ext in (("w1b", w1b_e), ("w2a", w2a_e), ("w2b", w2b_e)):
            t = wp.tile([128, 2, H], F32R, tag=nm)
            for k in range(2):
                nc.sync.dma_start(out=t[:, k, :], in_=ext[k])
            gin_w[nm] = t
        gin_b = {}
        for nm, ext in (
            ("b1a", b1a_e),
            ("b1b", b1b_e),
            ("b2a", b2a_e),
            ("b2b", b2b_e),
        ):
            t = wp.tile([128, 2], F32, tag=nm)
            nc.sync.dma_start(out=t[:, :], in_=ext[:, :])
            gin_b[nm] = t
        wa1c = wp.tile([128, 2, HA], BF16, tag="wa1c")
        wa1p = wp.tile([128, 2, HA], F32, tag="wa1p")
        wc1 = wp.tile([128, 2, HA], F32, tag="wc1")
        for t, ext in ((wa1c, wa1c_e), (wa1p, wa1p_e), (wc1, wc1_e)):
            for k in range(2):
                nc.sync.dma_start(out=t[:, k, :], in_=ext[k])
        wa1m = wp.tile([4, HA], BF16, tag="wa1m")
        nc.sync.dma_start(out=wa1m[:, :], in_=wa1m_e[:, :])
        wa2 = wp.tile([HA, HA], BF16, tag="wa2")
        nc.sync.dma_start(out=wa2[:, :], in_=wa2_e[:, :])
        wa3 = wp.tile([HA, 1], BF16, tag="wa3")
        nc.sync.dma_start(out=wa3[:, :], in_=wa3_e[:, :])
        wc2 = wp.tile([HA, HA], F32, tag="wc2")
        nc.sync.dma_start(out=wc2[:, :], in_=wc2_e[:, :])
        wc3 = wp.tile([HA, 1], F32, tag="wc3")
        nc.sync.dma_start(out=wc3[:, :], in_=wc3_e[:, :])
        ba1 = wp.tile([HA, 1], F32, tag="ba1")
        nc.sync.dma_start(out=ba1[:, :], in_=ba1_e[:, :])
        ba2 = wp.tile([HA, 1], F32, tag="ba2")
        nc.sync.dma_start(out=ba2[:, :], in_=ba2_e[:, :])
        bc1 = wp.tile([HA, 1], F32, tag="bc1")
        nc.sync.dma_start(out=bc1[:, :], in_=bc1_e[:, :])
        bc2 = wp.tile([HA, 1], F32, tag="bc2")
        nc.sync.dma_start(out=bc2[:, :], in_=bc2_e[:, :])
        bc3 = wp.tile([1, 1], F32, tag="bc3")
        nc.sync.dma_start(out=bc3[:, :], in_=bc3_e[:, :])

        for b in range(BPC):
            # ---- per-batch inputs (batch 0 core preloaded before weights)
            tl = preload if b == 0 else load_core(b)
            load_rest(b, tl)
            adjT, adjTt = tl["adjT"], tl["adjTt"]
            x_sb, x_tl, xT_sb = tl["x_sb"], tl["x_tl"], tl["xT_sb"]
            ST_sb, ST_tl = tl["ST_sb"], tl["ST_tl"]
            machT, mneg = tl["machT"], tl["mneg"]

            def adjT_blk(j, c0, cn):
                if j < 7:
                    return adjT[:, j, c0 : c0 + cn]
                return adjTt[0:104, c0 : c0 + cn]

            def x_blk(j):
                if j < 7:
                    return x_sb[:, j, :]
                return x_tl[0:104, :]

            def ST_blk(j):
                if j < 7:
                    return ST_sb[:, j, 0:SC]
                return ST_tl[0:104, 0:SC]

            # ---- GIN layer 1 --------------------------------------------
            # pooled1T[d, i] = sum_j x[j, d] * adjT[j, i]  (+ (1+eps0)*xT)
            p1T = hp.tile([D, N], F32R, tag="p1T")
            for c0, cn in CH:
                q = pmm.tile([D, 512], F32, tag="mm")
                for j in range(NT):
                    nc.tensor.matmul(
                        q[0:D, 0:cn],
                        lhsT=x_blk(j),
                        rhs=adjT_blk(j, c0, cn),
                        start=(j == 0),
                        stop=(j == NT - 1),
                    )
                nc.vector.scalar_tensor_tensor(
                    out=p1T[:, c0 : c0 + cn],
                    in0=xT_sb[:, c0 : c0 + cn],
                    scalar=eps0[0:D, :],
                    in1=q[0:D, 0:cn],
                    op0=ALU.mult,
                    op1=ALU.add,
                )

            # relu1T = relu(w1a^T @ p1T + b1a)
            r1T = hp.tile([128, 2, N], F32R, tag="r1T")
            for m in range(2):
                for c0, cn in CH:
                    q = pmm.tile([128, 512], F32, tag="mm")
                    nc.tensor.matmul(
                        q[:, 0:cn],
                        lhsT=w1a[:, m * 128 : (m + 1) * 128],
                        rhs=p1T[:, c0 : c0 + cn],
                        start=True,
                        stop=True,
                    )
                    nc.scalar.activation(
                        out=r1T[:, m, c0 : c0 + cn],
                        in_=q[:, 0:cn],
                        func=AF.Relu,
                        bias=gin_b["b1a"][:, m : m + 1],
                    )

            # h1T = relu(w1b^T @ r1T + b1b); h1 node-major via PE transpose
            h1T = hp.tile([128, 2, N], BF16, tag="h1T")
            for m in range(2):
                for c0, cn in CH:
                    q = pmm.tile([128, 512], F32, tag="mm")
                    for k in range(2):
                        nc.tensor.matmul(
                            q[:, 0:cn],
                            lhsT=gin_w["w1b"][:, k, m * 128 : (m + 1) * 128],
                            rhs=r1T[:, k, c0 : c0 + cn],
                            start=(k == 0),
                            stop=(k == 1),
                        )
                    nc.scalar.activation(
                        out=h1T[:, m, c0 : c0 + cn],
                        in_=q[:, 0:cn],
                        func=AF.Relu,
                        bias=gin_b["b1b"][:, m : m + 1],
                    )
            h1nm = hp.tile([128, NT, H], BF16, tag="h1nm")
            for m in range(2):
                for j in range(NT):
                    tsz = TS[j]
                    tq = ptp.tile([128, 128], BF16, tag="tp")
                    nc.tensor.transpose(
                        tq[0:tsz, 0:128],
                        in_=h1T[:, m, j * 128 : j * 128 + tsz],
                        identity=ident[:, :],
                    )
                    nc.vector.tensor_copy(
                        out=h1nm[0:tsz, j, m * 128 : (m + 1) * 128],
                        in_=tq[0:tsz, 0:128],
                    )

            # ---- GIN layer 2 --------------------------------------------
            p2T = hp.tile([128, 2, N], F32R, tag="p2T")
            for m in range(2):
                for c0, cn in CH:
                    q = pmm.tile([128, 512], F32, tag="mm")
                    for j in range(NT):
                        nc.tensor.matmul(
                            q[:, 0:cn],
                            lhsT=h1nm[0 : TS[j], j, m * 128 : (m + 1) * 128],
                            rhs=adjT_blk(j, c0, cn),
                            start=(j == 0),
                            stop=(j == NT - 1),
                        )
                    nc.vector.scalar_tensor_tensor(
                        out=p2T[:, m, c0 : c0 + cn],
                        in0=h1T[:, m, c0 : c0 + cn],
                        scalar=eps1[:, :],
                        in1=q[:, 0:cn],
                        op0=ALU.mult,
                        op1=ALU.add,
                    )

            r2T = hp.tile([128, 2, N], F32R, tag="r2T")
            for m in range(2):
                for c0, cn in CH:
                    q = pmm.tile([128, 512], F32, tag="mm")
                    for k in range(2):
                        nc.tensor.matmul(
                            q[:, 0:cn],
                            lhsT=gin_w["w2a"][:, k, m * 128 : (m + 1) * 128],
                            rhs=p2T[:, k, c0 : c0 + cn],
                            start=(k == 0),
                            stop=(k == 1),
                        )
                    nc.scalar.activation(
                        out=r2T[:, m, c0 : c0 + cn],
                        in_=q[:, 0:cn],
                        func=AF.Relu,
                        bias=gin_b["b2a"][:, m : m + 1],
                    )

            h2T = hp.tile([128, 2, N], BF16, tag="h2T")
            for m in range(2):
                for c0, cn in CH:
                    q = pmm.tile([128, 512], F32, tag="mm")
                    for k in range(2):
                        nc.tensor.matmul(
                            q[:, 0:cn],
                            lhsT=gin_w["w2b"][:, k, m * 128 : (m + 1) * 128],
                            rhs=r2T[:, k, c0 : c0 + cn],
                            start=(k == 0),
                            stop=(k == 1),
                        )
                    nc.scalar.activation(
                        out=h2T[:, m, c0 : c0 + cn],
                        in_=q[:, 0:cn],
                        func=AF.Relu,
                        bias=gin_b["b2b"][:, m : m + 1],
                    )
            h2nm = hp.tile([128, NT, H], BF16, tag="h2nm")
            for m in range(2):
                for j in range(NT):
                    tsz = TS[j]
                    tq = ptp.tile([128, 128], BF16, tag="tp")
                    nc.tensor.transpose(
                        tq[0:tsz, 0:128],
                        in_=h2T[:, m, j * 128 : j * 128 + tsz],
                        identity=ident[:, :],
                    )
                    nc.vector.tensor_copy(
                        out=h2nm[0:tsz, j, m * 128 : (m + 1) * 128],
                        in_=tq[0:tsz, 0:128],
                    )

            # ---- candidate gather + graph pool (one matmul) --------------
            # cfT[d, c] = sum_n h2[n, d] * ST[n, c]; col 100 = h_pooled
            cfT = hp.tile([128, 2, SC], BF16, tag="cfT")
            hp32 = hp.tile([128, 2], F32, tag="hp32")
            for m in range(2):
                q = ptp.tile([128, SC], F32, tag="tp")
                for j in range(NT):
                    nc.tensor.matmul(
                        q[:, 0:SC],
                        lhsT=h2nm[0 : TS[j], j, m * 128 : (m + 1) * 128],
                        rhs=ST_blk(j),
                        start=(j == 0),
                        stop=(j == NT - 1),
                    )
                nc.scalar.copy(out=cfT[:, m, :], in_=q[:, 0:SC])
                nc.scalar.copy(out=hp32[:, m : m + 1], in_=q[:, 100:101])

            # ---- actor bias u = wa1p^T @ h_pooled + ba1 ------------------
            qu = psm.tile([HA, 1], F32, tag="qu")
            for k in range(2):
                nc.tensor.matmul(
                    qu[:, :],
                    lhsT=wa1p[:, k, :],
                    rhs=hp32[:, k : k + 1],
                    start=(k == 0),
                    stop=(k == 1),
                )
            ua = hp.tile([HA, 1], F32, tag="ua")
            nc.vector.tensor_add(out=ua[:, :], in0=qu[:, :], in1=ba1[:, :])

            # ---- actor layer 1: cand(rep10) + mach + bias ----------------
            a1T = hp.tile([HA, N], BF16, tag="a1T")
            for c0, cn in CHA:
                q = pmm.tile([HA, 512], F32, tag="mm")
                for k in range(2):
                    src = cfT[:, k, c0 // NM : (c0 + cn) // NM]
                    nc.tensor.matmul(
                        q[:, 0:cn],
                        lhsT=wa1c[:, k, :],
                        rhs=_rep10_ap(src),
                        start=(k == 0),
                        stop=False,
                    )
                nc.tensor.matmul(
                    q[:, 0:cn],
                    lhsT=wa1m[:, :],
                    rhs=machT[:, c0 : c0 + cn],
                    start=False,
                    stop=True,
                )
                nc.scalar.activation(
                    out=a1T[:, c0 : c0 + cn],
                    in_=q[:, 0:cn],
                    func=AF.Tanh,
                    bias=ua[:, :],
                )

            # ---- actor layer 2 ------------------------------------------
            a2T = hp.tile([HA, N], BF16, tag="a2T")
            for c0, cn in CHA:
                q = pmm.tile([HA, 512], F32, tag="mm")
                nc.tensor.matmul(
                    q[:, 0:cn],
                    lhsT=wa2[:, :],
                    rhs=a1T[:, c0 : c0 + cn],
                    start=True,
                    stop=True,
                )
                nc.scalar.activation(
                    out=a2T[:, c0 : c0 + cn],
                    in_=q[:, 0:cn],
                    func=AF.Tanh,
                    bias=ba2[:, :],
                )

            # ---- scores + mask (+ba3 folded into maskneg) ----------------
            sT = hp.tile([1, N], F32, tag="sT")
            for c0, cn in CHA:
                q = pmm.tile([1, 512], F32, tag="mm")
                nc.tensor.matmul(
                    q[0:1, 0:cn],
                    lhsT=wa3[:, :],
                    rhs=a2T[:, c0 : c0 + cn],
                    start=True,
                    stop=True,
                )
                nc.vector.tensor_add(
                    out=sT[:, c0 : c0 + cn],
                    in0=q[0:1, 0:cn],
                    in1=mneg[:, c0 : c0 + cn],
                )

            # ---- masked softmax over the 1000 candidates -----------------
            nmx = hp.tile([1, 1], F32, tag="nmx")
            nc.vector.reduce_max(out=nmx[:, :], in_=sT[:, :], axis=mybir.AxisListType.X, negate=True)
            esb = hp.tile([1, N], F32, tag="esb")
            ssum = hp.tile([1, 1], F32, tag="ssum")
            nc.scalar.activation(
                out=esb[:, :],
                in_=sT[:, :],
                func=AF.Exp,
                bias=nmx[:, :],
                accum_out=ssum[:, :],
            )
            rsum = hp.tile([1, 1], F32, tag="rsum")
            nc.vector.reciprocal(out=rsum[:, :], in_=ssum[:, :])
            pi = hp.tile([1, N], F32, tag="pi")
            nc.vector.tensor_scalar_mul(pi[:, :], in0=esb[:, :], scalar1=rsum[:, :])
            nc.sync.dma_start(out=out_e[b : b + 1, 0:1000], in_=pi[:, :])

            # ---- critic head --------------------------------------------
            qc1 = psm.tile([HA, 1], F32, tag="qu")
            for k in range(2):
                nc.tensor.matmul(
                    qc1[:, :],
                    lhsT=wc1[:, k, :],
                    rhs=hp32[:, k : k + 1],
                    start=(k == 0),
                    stop=(k == 1),
                )
            c1 = hp.tile([HA, 1], F32, tag="c1")
            nc.scalar.activation(out=c1[:, :], in_=qc1[:, :], func=AF.Tanh, bias=bc1[:, :])
            qc2 = psm.tile([HA, 1], F32, tag="qu")
            nc.tensor.matmul(qc2[:, :], lhsT=wc2[:, :], rhs=c1[:, :], start=True, stop=True)
            c2 = hp.tile([HA, 1], F32, tag="c2")
            nc.scalar.activation(out=c2[:, :], in_=qc2[:, :], func=AF.Tanh, bias=bc2[:, :])
            qv = psm.tile([1, 1], F32, tag="qu")
            nc.tensor.matmul(qv[:, :], lhsT=wc3[:, :], rhs=c2[:, :], start=True, stop=True)
            v = hp.tile([1, 1], F32, tag="v")
            nc.scalar.activation(out=v[:, :], in_=qv[:, :], func=AF.Identity, bias=bc3[:, :])
            nc.sync.dma_start(out=out_e[b : b + 1, 1000:1001], in_=v[:, :])

    _split_sync_waits(nc)
    return nc


_NC_CACHE = {}


def _get_nc():
    if "nc" not in _NC_CACHE:
        _NC_CACHE["nc"] = _build_nc()
    return _NC_CACHE["nc"]


def _leaf(a):
    return np.asarray(a)


def _prep_inputs(inputs):
    x = _leaf(inputs["x"]).astype(np.float32)
    adj = _leaf(inputs["adj_matrix"]).astype(np.float32)
    gpool = _leaf(inputs["graph_pool"]).astype(np.float32)
    cand = _leaf(inputs["candidate"])
    mask = _leaf(inputs["mask"])
    mach = _leaf(inputs["machine_feat"]).astype(np.float32)
    gin_params = [[(_leaf(w), _leaf(bb)) for (w, bb) in layer] for layer in inputs["gin_params"]]
    eps = _leaf(inputs["eps"]).astype(np.float32)
    actor = [(_leaf(w), _leaf(bb)) for (w, bb) in inputs["actor_params"]]
    critic = [(_leaf(w), _leaf(bb)) for (w, bb) in inputs["critic_params"]]

    # torch.unique semantics (jnp.unique size=NJ fill=0): sorted unique,
    # truncated/padded to NJ
    cand0 = cand[:, :, 0].astype(np.int64)
    cand_ops = np.zeros((B, NJ), np.int64)
    for bb in range(B):
        u = np.unique(cand0[bb])
        if len(u) >= NJ:
            cand_ops[bb] = u[:NJ]
        else:
            cand_ops[bb, : len(u)] = u
    # one-hot gather matrix, graph_pool packed as column 100
    ST = np.zeros((B, N, SC), np.float32)
    bidx = np.repeat(np.arange(B), NJ)
    ST[bidx, cand_ops.reshape(-1), np.tile(np.arange(NJ), B)] = 1.0
    ST[:, :, 100] = gpool

    ba3 = float(np.asarray(actor[2][1]).reshape(-1)[0])
    maskneg = np.where(mask, np.float32(NEG), np.float32(0.0)).astype(np.float32) + ba3

    shared = {
        "eps": eps.reshape(2, 1),
        "w1a": gin_params[0][0][0].astype(np.float32),
        "b1a": np.ascontiguousarray(
            gin_params[0][0][1].astype(np.float32).reshape(2, 128).T
        ),
        "w1b": gin_params[0][1][0].astype(np.float32).reshape(2, 128, H),
        "b1b": np.ascontiguousarray(
            gin_params[0][1][1].astype(np.float32).reshape(2, 128).T
        ),
        "w2a": gin_params[1][0][0].astype(np.float32).reshape(2, 128, H),
        "b2a": np.ascontiguousarray(
            gin_params[1][0][1].astype(np.float32).reshape(2, 128).T
        ),
        "w2b": gin_params[1][1][0].astype(np.float32).reshape(2, 128, H),
        "b2b": np.ascontiguousarray(
            gin_params[1][1][1].astype(np.float32).reshape(2, 128).T
        ),
        "wa1c": np.ascontiguousarray(actor[0][0][0:256]).astype(_nbf16).reshape(2, 128, HA),
        "wa1m": np.ascontiguousarray(actor[0][0][256:260]).astype(_nbf16),
        "wa1p": np.ascontiguousarray(actor[0][0][260:516]).astype(np.float32).reshape(2, 128, HA),
        "ba1": actor[0][1].astype(np.float32).reshape(HA, 1),
        "wa2": actor[1][0].astype(_nbf16),
        "ba2": actor[1][1].astype(np.float32).reshape(HA, 1),
        "wa3": actor[2][0].astype(_nbf16),
        "wc1": critic[0][0].astype(np.float32).reshape(2, 128, HA),
        "bc1": critic[0][1].astype(np.float32).reshape(HA, 1),
        "wc2": critic[1][0].astype(np.float32),
        "bc2": critic[1][1].astype(np.float32).reshape(HA, 1),
        "wc3": critic[2][0].astype(np.float32),
        "bc3": critic[2][1].astype(np.float32).reshape(1, 1),
    }

    adj_bf = adj.astype(_nbf16)
    x_bf = x.astype(_nbf16)
    mach_bf = mach.astype(_nbf16)
    ST_bf = ST.astype(_nbf16)

    in_maps = []
    for i in range(N_CORES):
        sl = slice(i * BPC, (i + 1) * BPC)
        m = dict(shared)
        m["adjT"] = np.ascontiguousarray(adj_bf[sl].transpose(0, 2, 1))
        m["x"] = np.ascontiguousarray(x_bf[sl])
        m["xT"] = np.ascontiguousarray(x_bf[sl].transpose(0, 2, 1))
        m["ST"] = np.ascontiguousarray(ST_bf[sl])
        m["machT"] = np.ascontiguousarray(mach_bf[sl].transpose(0, 2, 1))
        m["maskneg"] = np.ascontiguousarray(maskneg[sl].reshape(BPC, 1, N))
        in_maps.append(m)
    return in_maps


def _run(inputs, trace=False):
    in_maps = _prep_inputs(inputs)
    nc = _get_nc()
    res = run_bass_kernel_spmd(
        nc, in_maps, core_ids=list(range(N_CORES)), trace=trace
    )
    outs = np.concatenate([np.asarray(res.results[i]["out"]) for i in range(N_CORES)], axis=0)
    pi = outs[:, 0:1000].reshape(B, N, 1).astype(np.float32)
    v = outs[:, 1000:1001].astype(np.float32)
    return pi, v, res.exec_time_ns


def kernel(**inputs):
    pi, v, _ = _run(inputs, trace=False)
    return pi, v
